# revision 17
# baseline (speedup 1.0000x reference)
"""Trainium2 Bass kernel for nn_DepthPriorLoss (RANSAC depth-prior alignment).

Contract: kernel(**inputs) takes the FULL inputs from setup_inputs() and
returns the FULL outputs of reference():
    (total_loss, target_inv_ren [1024,2048], prior_metric_depth [1024,2048])

Strategy (8 NeuronCores, SPMD):
  * Host (exact, cheap): mask/num_pts, RANSAC random pairs (jax CPU threefry,
    bit-exact with the reference), scales/shifts, the MAD threshold `dyn`
    (jnp.nanmedian on CPU, bit-exact), survivor (s>0) pruning.
  * Device: the O(50M) inlier-count matrix, sharded over points (each core
    evaluates all surviving (s,t) candidates against SUB_N/8 points), via a
    K=3 fp32 PE matmul r = s*x + t - y and fused DVE/ACT count-accumulate;
    AllReduce of counts; on-device first-argmax -> (s,t); and the per-pixel
    maps 1/(d+1e-6), 1/max(s*d_pri+t, 1e-4) plus the masked-L1 partial sums,
    sharded over rows.
  * Host: final scalar assembly of the loss.
"""
import os
import numpy as np

H, W = 1024, 2048
NCORES = 8
ITERATIONS = 1000
SUB_N = 50000
THRESH = 0.01
LAMBDA_L1 = 0.5

PPC = SUB_N // NCORES          # points per core (6250)
CH = 512                       # point-chunk (PSUM bank / fp32 moving max)
NCH = (PPC + CH - 1) // CH     # 13
PPC_PAD = NCH * CH             # 6656
ROWS = H // NCORES             # 128 pixel rows per core
PIXCH = 1024
NPIX = W // PIXCH              # 2

X_PAD = np.float32(0.0)
Y_PAD = np.float32(1.5e38)
T_PAD = np.float32(-1.5e38)
IDX_PAD = np.float32(2.0e9)

def _chop(v, keep=10):
    b = v.view(np.uint32) & np.uint32(0xFFFFFFFF ^ ((1 << (23 - keep)) - 1))
    return b.view(np.float32)


def _split3(v):
    v = np.ascontiguousarray(v, np.float32)
    v0 = _chop(v)
    v1 = _chop((v - v0).astype(np.float32))
    v2 = _chop((v - v0 - v1).astype(np.float32))
    return v0, v1, v2


LAST_PROFILE = {}
LAST_DEBUG = {}
_PROGRAM_CACHE = {}


def _import_bass():
    import sys
    try:
        import concourse.bass  # noqa: F401
    except ImportError:
        for p in ("/opt/trn_rl_repo", "/root/.axon_site/_ro/trn_rl_repo"):
            if os.path.isdir(p) and p not in sys.path:
                sys.path.insert(0, p)
    import concourse.bass as bass
    import concourse.mybir as mybir
    import concourse.tile as tile
    import concourse.bacc as bacc
    return bass, mybir, tile, bacc


DVE_CHUNKS = frozenset(range(1, NCH, 2))  # 6 of 13 chunks on DVE

def _build_program(S, dyn, mm_dtype="f32r12", single_core=False):
    """Build the SPMD Bass program. S = padded survivor count (mult of 128)."""
    bass, mybir, tile, bacc = _import_bass()
    F32 = mybir.dt.float32
    BF16 = mybir.dt.bfloat16
    Alu = mybir.AluOpType
    Act = mybir.ActivationFunctionType
    AX = mybir.AxisListType

    NT = S // 128
    nc = bacc.Bacc("TRN2", target_bir_lowering=False, debug=False,
                   num_devices=1 if single_core else NCORES)

    # ---- I/O ----
    if mm_dtype == "f32r12":
        MMDT, KK = mybir.dt.float32r, 12
    else:
        MMDT, KK = F32, 3
    i_lhsT = nc.dram_tensor("lhsT", [KK, S], MMDT, kind="ExternalInput").ap()
    i_rhs = nc.dram_tensor("rhs3", [KK, PPC_PAD], MMDT, kind="ExternalInput").ap()
    i_idx = nc.dram_tensor("idxrow", [1, S], F32, kind="ExternalInput").ap()
    i_s = nc.dram_tensor("srow", [1, S], F32, kind="ExternalInput").ap()
    i_t = nc.dram_tensor("trow", [1, S], F32, kind="ExternalInput").ap()
    i_dren = nc.dram_tensor("dren", [ROWS, W], F32, kind="ExternalInput").ap()
    i_dpri = nc.dram_tensor("dpri", [ROWS, W], F32, kind="ExternalInput").ap()
    o_tinv = nc.dram_tensor("tinv", [ROWS, W], F32, kind="ExternalOutput").ap()
    o_met = nc.dram_tensor("metric", [ROWS, W], F32, kind="ExternalOutput").ap()
    o_l1p = nc.dram_tensor("l1p", [ROWS, 1], F32, kind="ExternalOutput").ap()
    o_cnt = nc.dram_tensor("cnts", [1, S], F32, kind="ExternalOutput").ap()

    cc_in = nc.dram_tensor("cc_in", [S], F32)
    cc_out = nc.dram_tensor("cc_out", [S], F32, addr_space="Shared")
    st_b = nc.dram_tensor("st_bounce", [1, 2], F32)

    with tile.TileContext(nc) as tc:
        with (
            tc.tile_pool(name="per", bufs=1) as per,      # persistent
            tc.tile_pool(name="scr", bufs=2) as scr,      # rotating scratch
            tc.tile_pool(name="ps", bufs=4, space="PSUM") as ps,
        ):
            # ---- persistent tiles / constants ----
            lhsT = per.tile([KK, S], MMDT)
            rhs = per.tile([KK, PPC_PAD], MMDT)
            nc.sync.dma_start(lhsT[:], i_lhsT[:])
            nc.sync.dma_start(rhs[:], i_rhs[:])

            dyn_tile = per.tile([128, CH], F32)
            nc.vector.memset(dyn_tile[:], dyn)
            dyn_col = per.tile([128, 1], F32)
            nc.vector.memset(dyn_col[:], dyn)
            eps_col = per.tile([128, 1], F32)
            zero_col = per.tile([128, 1], F32)

            dren = per.tile([ROWS, W], F32)
            dpri = per.tile([ROWS, W], F32)
            tinv = per.tile([ROWS, W], F32)
            met = per.tile([ROWS, W], F32)
            nc.sync.dma_start(dren[:], i_dren[:])
            nc.sync.dma_start(dpri[:], i_dpri[:])

            # ---- count phase ----
            cnt_cols = [per.tile([128, 2 * NCH], F32, tag=f"cntc{t}", name=f"cntc{t}") for t in range(NT)]
            sg_cols = [per.tile([128, 2 * NCH], F32, tag=f"sgc{t}", name=f"sgc{t}") for t in range(NT)]
            for t in range(NT):
                nc.vector.memset(cnt_cols[t][:], 0.0)
                nc.vector.memset(sg_cols[t][:], 0.0)

            n_dve_chunks = len([c for c in range(NCH) if c in DVE_CHUNKS])
            for ch in range(NCH):
                for t in range(NT):
                    r_ps = ps.tile([128, CH], F32, tag="r")
                    nc.tensor.matmul(r_ps[:], lhsT[:, bass.ts(t, 128)],
                                     rhs[:, bass.ts(ch, CH)])
                    if ch not in DVE_CHUNKS:
                        # ACT path: net counts via the sign trick
                        sg1 = scr.tile([128, CH], BF16, tag="sg")
                        nc.scalar.activation(
                            sg1[:], r_ps[:], Act.Sign, bias=dyn_col[:],
                            scale=-1.0,
                            accum_out=sg_cols[t][:, 2 * ch:2 * ch + 1])
                        sg2 = scr.tile([128, CH], BF16, tag="sg")
                        nc.scalar.activation(
                            sg2[:], r_ps[:], Act.Sign, bias=dyn_col[:],
                            scale=1.0,
                            accum_out=sg_cols[t][:, 2 * ch + 1:2 * ch + 2])
                    else:
                        # DVE path: two one-sided counts (A=#(r<dyn), B=#(r>-dyn))
                        cb1 = scr.tile([128, CH], BF16, tag="cb1")
                        nc.vector.tensor_scalar(
                            out=cb1[:], in0=r_ps[:], scalar1=dyn, scalar2=None,
                            op0=Alu.is_lt, op1=Alu.add,
                            accum_out=cnt_cols[t][:, 2 * ch:2 * ch + 1])
                        cb2 = scr.tile([128, CH], BF16, tag="cb2")
                        nc.vector.tensor_scalar(
                            out=cb2[:], in0=r_ps[:], scalar1=-dyn, scalar2=None,
                            op0=Alu.is_gt, op1=Alu.add,
                            accum_out=cnt_cols[t][:, 2 * ch + 1:2 * ch + 2])

            # Serialize ACT table sets: pixel Ln/Exp wait on the sign phase
            # via a real data dep (eps/zero cols derive from sg_cols).
            nc.vector.tensor_scalar(zero_col[:], sg_cols[0][:, 0:1], 0.0,
                                    None, op0=Alu.mult)
            nc.vector.tensor_scalar(eps_col[:], zero_col[:], 1e-6,
                                    None, op0=Alu.add)

            # per-tile counts = sum(cnt_cols) + floor(0.5*sum(sg_cols))
            for t in range(NT):
                c1 = scr.tile([128, 1], F32, tag="c1")
                nc.vector.reduce_sum(c1[:], cnt_cols[t][:], axis=AX.X)
                c2 = scr.tile([128, 1], F32, tag="c2")
                nc.vector.reduce_sum(c2[:], sg_cols[t][:], axis=AX.X)
                hf = scr.tile([128, 1], F32, tag="hf")
                nc.vector.tensor_scalar(hf[:], c2[:], 0.5, None, op0=Alu.mult)
                # floor(hf) for hf in {k, k+0.5}: RNE((hf-0.25)+2^23)-2^23
                md = scr.tile([128, 1], F32, tag="md")
                nc.vector.tensor_scalar(md[:], hf[:], 8388607.75, None, op0=Alu.add)
                fl = scr.tile([128, 1], F32, tag="fl")
                nc.vector.tensor_scalar(fl[:], md[:], -8388608.0, None, op0=Alu.add)
                c1b = scr.tile([128, 1], F32, tag="c1b")
                nc.vector.tensor_scalar(c1b[:], c1[:], float(-n_dve_chunks * CH),
                                        None, op0=Alu.add)
                ct = scr.tile([128, 1], F32, tag="ct")
                nc.vector.tensor_add(ct[:], c1b[:], fl[:])
                nc.sync.dma_start(
                    bass.AP(cc_in, t * 128, [[1, 128]]), ct[:])

            # ---- all-reduce counts across the 8 cores ----
            if single_core:
                nc.gpsimd.dma_start(cc_out[:], cc_in[:])
            else:
                nc.gpsimd.collective_compute(
                    "AllReduce", Alu.add,
                    replica_groups=[list(range(NCORES))],
                    ins=[cc_in[:]],
                    outs=[cc_out[:]],
                )

            # ---- argmax (first max, original iteration order) ----
            crow = per.tile([1, S], F32)
            nc.sync.dma_start(crow[:], bass.AP(cc_out, 0, [[0, 1], [1, S]]))
            nc.sync.dma_start(o_cnt[:], crow[:])
            irow = per.tile([1, S], F32)
            srow = per.tile([1, S], F32)
            trow = per.tile([1, S], F32)
            nc.sync.dma_start(irow[:], i_idx[:])
            nc.sync.dma_start(srow[:], i_s[:])
            nc.sync.dma_start(trow[:], i_t[:])

            mx = per.tile([1, 1], F32)
            nc.vector.reduce_max(mx[:], crow[:], axis=AX.X)
            nm = scr.tile([1, S], F32, tag="nm")
            nc.vector.tensor_scalar(nm[:], crow[:], mx[:, 0:1], 4.0e9,
                                    op0=Alu.not_equal, op1=Alu.mult)
            cand = scr.tile([1, S], F32, tag="cand")
            nc.vector.tensor_add(cand[:], nm[:], irow[:])
            best = per.tile([1, 1], F32)
            nc.vector.tensor_reduce(best[:], cand[:], axis=AX.X, op=Alu.min)

            oh1 = scr.tile([1, S], F32, tag="oh1")
            s_best = per.tile([1, 1], F32)
            nc.vector.scalar_tensor_tensor(
                out=oh1[:], in0=irow[:], scalar=best[:, 0:1], in1=srow[:],
                op0=Alu.is_equal, op1=Alu.mult, accum_out=s_best[:])
            oh2 = scr.tile([1, S], F32, tag="oh2")
            t_best = per.tile([1, 1], F32)
            nc.vector.scalar_tensor_tensor(
                out=oh2[:], in0=irow[:], scalar=best[:, 0:1], in1=trow[:],
                op0=Alu.is_equal, op1=Alu.mult, accum_out=t_best[:])

            # broadcast (s,t) to all 128 partitions via a DRAM bounce
            nc.sync.dma_start(bass.AP(st_b, 0, [[1, 1], [1, 1]]), s_best[:])
            nc.sync.dma_start(bass.AP(st_b, 1, [[1, 1], [1, 1]]), t_best[:])
            s_bc = per.tile([128, 1], F32)
            t_bc = per.tile([128, 1], F32)
            nc.sync.dma_start(s_bc[:], bass.AP(st_b, 0, [[0, 128], [1, 1]]))
            nc.sync.dma_start(t_bc[:], bass.AP(st_b, 1, [[0, 128], [1, 1]]))

            # ---- pixel phase ----
            # t_inv = exp(-ln(d_ren + 1e-6))   (independent of s,t)
            # phase 1: all Ln ops batched (one table set), aligned prep
    

            lndT = []
            lnmM = []
            alT = []
            for pc in range(NPIX):
                cs = bass.ts(pc, PIXCH)
                lnd = scr.tile([128, PIXCH], F32, tag=f"lndT{pc}", bufs=1,
                               name=f"lndT{pc}")
                nc.scalar.activation(lnd[:], dren[:, cs], Act.Ln,
                                     bias=eps_col[:], scale=1.0)
                lndT.append(lnd)
                v = scr.tile([128, PIXCH], F32, tag=f"alv{pc}", bufs=1,
                             name=f"alv{pc}")
                nc.vector.tensor_scalar(v[:], dpri[:, cs], s_bc[:, 0:1], None,
                                        op0=Alu.mult)
                nc.vector.tensor_scalar(v[:], v[:], t_bc[:, 0:1], None,
                                        op0=Alu.add)
                alT.append(v)
                mx2 = scr.tile([128, PIXCH], F32, tag="mx2")
                nc.vector.tensor_scalar(mx2[:], v[:], 1e-4, None, op0=Alu.max)
                lnm = scr.tile([128, PIXCH], F32, tag=f"lnmM{pc}", bufs=1,
                               name=f"lnmM{pc}")
                nc.scalar.activation(lnm[:], mx2[:], Act.Ln,
                                     bias=zero_col[:], scale=1.0)
                lnmM.append(lnm)
            # Exp ops wait for every Ln via a rebuilt bias column
            zeroE = per.tile([128, 1], F32)
            nc.vector.tensor_scalar(zeroE[:], lnmM[-1][:, 0:1], 0.0, None,
                                    op0=Alu.mult)

            l1cols = per.tile([128, NPIX], F32)
            for pc in range(NPIX):
                cs = bass.ts(pc, PIXCH)
                al = alT[pc]
                nc.scalar.activation(tinv[:, cs], lndT[pc][:], Act.Exp,
                                     bias=zeroE[:], scale=-1.0)
                nc.scalar.activation(met[:, cs], lnmM[pc][:], Act.Exp,
                                     bias=zeroE[:], scale=-1.0)
                # masked L1 partial: |al - tinv| * (dren>0.1) * (dren<100)
                dm = scr.tile([128, PIXCH], F32, tag="dm")
                nc.vector.tensor_sub(dm[:], al[:], tinv[:, cs])
                ab = dm
                I32 = mybir.dt.int32
                nc.vector.tensor_scalar(ab[:].bitcast(I32), dm[:].bitcast(I32),
                                        0x7FFFFFFF, None, op0=Alu.bitwise_and)
                m1 = scr.tile([128, PIXCH], F32, tag="m1")
                nc.vector.scalar_tensor_tensor(
                    out=m1[:], in0=dren[:, cs], scalar=0.1, in1=ab[:],
                    op0=Alu.is_gt, op1=Alu.mult)
                m2 = scr.tile([128, PIXCH], F32, tag="m2")
                nc.vector.scalar_tensor_tensor(
                    out=m2[:], in0=dren[:, cs], scalar=100.0, in1=m1[:],
                    op0=Alu.is_lt, op1=Alu.mult,
                    accum_out=l1cols[:, pc:pc + 1])
            nc.sync.dma_start(o_tinv[:], tinv[:])

            nc.sync.dma_start(o_met[:], met[:])
            l1p = per.tile([128, 1], F32)
            nc.vector.reduce_sum(l1p[:], l1cols[:], axis=AX.X)
            nc.sync.dma_start(o_l1p[:], l1p[:])

    nc.compile()
    return nc


def _host_control(rendered_depth, prior_disparity):
    """Exact host-side replication of the data-dependent control scalars."""
    import jax
    import jax.numpy as jnp
    cpu = jax.devices("cpu")[0]

    d_ren = np.asarray(rendered_depth, dtype=np.float32)
    d_pri = np.asarray(prior_disparity, dtype=np.float32)
    mask = (d_ren > 0.1) & (d_ren < 100.0) & np.isfinite(d_ren)
    mask_flat = mask.reshape(-1)
    num_pts = int(mask_flat.sum())
    n_valid = num_pts  # same expression in the reference
    P = d_ren.size

    x = d_pri.reshape(-1).astype(np.float32)
    y = (np.float32(1.0) / (d_ren.reshape(-1).astype(np.float32) + np.float32(1e-6)))

    idx_m = np.flatnonzero(mask_flat)
    if idx_m.size < P:
        idx_m = np.concatenate([idx_m, np.zeros(P - idx_m.size, idx_m.dtype)])

    with jax.default_device(cpu):
        k_pair, k_sub = jax.random.split(jax.random.key(42))
        pos = np.asarray(jax.random.randint(k_pair, (ITERATIONS, 2), 0, num_pts))
        spos = np.asarray(jax.random.randint(k_sub, (SUB_N,), 0, num_pts))
        y_nan = jnp.where(jnp.asarray(mask_flat), jnp.asarray(y), jnp.nan)
        med = jnp.nanmedian(y_nan)
        dynj = jnp.nanmedian(jnp.abs(y_nan - med)) * 0.5
        dynj = jnp.where(dynj < 1e-5, THRESH, dynj)
        dyn = np.float32(np.asarray(dynj))

    pi = idx_m[pos]
    x1, x2 = x[pi[:, 0]], x[pi[:, 1]]
    y1, y2 = y[pi[:, 0]], y[pi[:, 1]]
    scales = ((y2 - y1) / ((x2 - x1) + np.float32(1e-8))).astype(np.float32)
    shifts = (y1 - scales * x1).astype(np.float32)

    si = idx_m[spos]
    x_sub = x[si].astype(np.float32)
    y_sub = y[si].astype(np.float32)

    surv = np.flatnonzero(scales > 0)
    fallback = (num_pts < 10) or (surv.size == 0)
    if fallback:
        surv_s = np.ones(1, np.float32)
        surv_t = np.zeros(1, np.float32)
        surv_i = np.zeros(1, np.float32)
    else:
        surv_s = scales[surv].astype(np.float32)
        surv_t = shifts[surv].astype(np.float32)
        surv_i = surv.astype(np.float32)

    return dict(d_ren=d_ren, d_pri=d_pri, num_pts=num_pts, n_valid=n_valid,
                dyn=dyn, scales=scales, shifts=shifts, x_sub=x_sub,
                y_sub=y_sub, surv_s=surv_s, surv_t=surv_t, surv_i=surv_i,
                fallback=fallback)


def _prepare(rendered_depth, prior_disparity, mm_dtype="f32r"):
    """Host control + program build + per-core input maps."""
    hc = _host_control(rendered_depth, prior_disparity)
    return _make_inputs(hc, mm_dtype) + (hc,)


def _make_inputs(hc, mm_dtype="f32r12"):
    dyn = float(hc["dyn"])

    # pad survivors to a multiple of 128
    S_real = hc["surv_s"].size
    NT = max(1, (S_real + 127) // 128)
    S = NT * 128
    s_full = np.ones(S, np.float32)
    t_full = np.full(S, T_PAD, np.float32)
    s_full[:S_real] = hc["surv_s"]
    t_full[:S_real] = hc["surv_t"]
    if mm_dtype == "f32r12":
        s0, s1, s2 = _split3(s_full)
        t0, t1, t2 = _split3(t_full)
        mone = -np.ones(S, np.float32)
        lhsT = np.stack([s0, s0, s1, s1, s0, s2, t0, t1, t2, mone, mone, mone])
    else:
        lhsT = np.stack([s_full, t_full, -np.ones(S, np.float32)])
    idxrow = np.full((1, S), IDX_PAD, np.float32)
    idxrow[0, :S_real] = hc["surv_i"]
    srow = np.ones((1, S), np.float32)
    srow[0, :S_real] = hc["surv_s"]
    trow = np.zeros((1, S), np.float32)
    trow[0, :S_real] = hc["surv_t"]

    key = (S, float(dyn), mm_dtype)
    if key not in _PROGRAM_CACHE:
        _PROGRAM_CACHE[key] = _build_program(S, dyn, mm_dtype)
    nc = _PROGRAM_CACHE[key]

    # per-core inputs
    in_maps = []
    for c in range(NCORES):
        xs = np.full(PPC_PAD, X_PAD, np.float32)
        ys = np.full(PPC_PAD, Y_PAD, np.float32)
        xs[:PPC] = hc["x_sub"][c * PPC:(c + 1) * PPC]
        ys[:PPC] = hc["y_sub"][c * PPC:(c + 1) * PPC]
        one = np.ones(PPC_PAD, np.float32)
        if mm_dtype == "f32r12":
            x0, x1, x2 = _split3(xs)
            y0, y1, y2 = _split3(ys)
            rhs3 = np.stack([x0, x1, x0, x1, x2, x0, one, one, one, y0, y1, y2])
        else:
            rhs3 = np.stack([xs, one, ys])
        rs = slice(c * ROWS, (c + 1) * ROWS)
        in_maps.append({
            "lhsT": lhsT, "rhs3": rhs3, "idxrow": idxrow,
            "srow": srow, "trow": trow,
            "dren": np.ascontiguousarray(hc["d_ren"][rs]),
            "dpri": np.ascontiguousarray(hc["d_pri"][rs]),
        })
    return nc, in_maps


def _finalize(results, hc):
    t_inv = np.concatenate([results[c]["tinv"] for c in range(NCORES)], axis=0)
    metric = np.concatenate([results[c]["metric"] for c in range(NCORES)], axis=0)
    l1_sum = float(sum(results[c]["l1p"].astype(np.float64).sum()
                       for c in range(NCORES)))
    n_valid = hc["n_valid"]
    l1 = l1_sum / max(n_valid, 1)
    total = np.float32(LAMBDA_L1 * l1)
    if n_valid < 100:
        total = np.float32(0.0)
    LAST_DEBUG.clear()
    LAST_DEBUG["cnts"] = results[0]["cnts"][0]
    LAST_DEBUG["hc"] = hc
    return total, t_inv, metric


def _exact_argmax_host(hc):
    """Reference-exact RANSAC winner, computed on host (fallback path)."""
    scales, shifts = hc["scales"], hc["shifts"]
    x_sub, y_sub, dyn = hc["x_sub"], hc["y_sub"], np.float32(hc["dyn"])
    counts = np.zeros(ITERATIONS, np.int64)
    CHh = 64
    for i0 in range(0, ITERATIONS, CHh):
        ss = scales[i0:i0 + CHh, None]
        tt = shifts[i0:i0 + CHh, None]
        w = ((ss * x_sub[None, :]).astype(np.float32) + tt).astype(np.float32)
        res = np.abs((w - y_sub[None, :]).astype(np.float32))
        counts[i0:i0 + CHh] = (res < dyn).sum(axis=1)
    cm = np.where(scales > 0, counts, -1)
    best = int(np.argmax(cm))
    if cm[best] >= 0 and hc["num_pts"] >= 10:
        return np.float32(scales[best]), np.float32(shifts[best])
    return np.float32(1.0), np.float32(0.0)


MARGIN_MIN = 64.0


def kernel(rendered_depth, prior_disparity):
    _import_bass()
    from concourse.bass_utils import run_bass_kernel_spmd

    hc = _host_control(rendered_depth, prior_disparity)
    nc, in_maps = _make_inputs(hc, "f32r12")
    trace = bool(os.environ.get("DEPTH_KERNEL_TRACE"))
    if trace:
        try:
            from antenv.axon_hooks import get_axon_ntff_profile_hook  # noqa: F401
        except ImportError:
            trace = False
    res = run_bass_kernel_spmd(nc, in_maps, list(range(NCORES)), trace=trace)
    LAST_PROFILE.clear()
    LAST_PROFILE["exec_time_ns"] = res.exec_time_ns
    LAST_PROFILE["res"] = res

    # Robustness certificate: the fp32r count matrix can deviate from the
    # reference-exact counts by a few units per iteration near the threshold.
    # If the observed top-2 margin is not comfortably larger than that, fall
    # back to the host-exact winner and rerun the (cheap) pixel phase with a
    # single forced candidate.
    S_real = int(hc["surv_s"].size)
    if not hc["fallback"] and S_real > 1:
        cnts = np.sort(res.results[0]["cnts"][0, :S_real])[::-1]
        margin_ok = bool(np.isfinite(cnts).all()) and \
            float(cnts[0] - cnts[1]) >= MARGIN_MIN
        if not margin_ok:
            s_b, t_b = _exact_argmax_host(hc)
            hc = dict(hc)
            hc["surv_s"] = np.array([s_b], np.float32)
            hc["surv_t"] = np.array([t_b], np.float32)
            hc["surv_i"] = np.zeros(1, np.float32)
            nc, in_maps = _make_inputs(hc, "f32r12")
            res = run_bass_kernel_spmd(nc, in_maps, list(range(NCORES)),
                                       trace=trace)
    return _finalize(res.results, hc)


# revision 21
# speedup vs baseline: 1.0166x; 1.0166x over previous
"""Trainium2 Bass kernel for nn_DepthPriorLoss (RANSAC depth-prior alignment).

Contract: kernel(**inputs) takes the FULL inputs from setup_inputs() and
returns the FULL outputs of reference():
    (total_loss, target_inv_ren [1024,2048], prior_metric_depth [1024,2048])

Strategy (8 NeuronCores, SPMD):
  * Host (exact, cheap): mask/num_pts, RANSAC random pairs (jax CPU threefry,
    bit-exact with the reference), scales/shifts, the MAD threshold `dyn`
    (jnp.nanmedian on CPU, bit-exact), survivor (s>0) pruning.
  * Device: the O(50M) inlier-count matrix, sharded over points (each core
    evaluates all surviving (s,t) candidates against SUB_N/8 points), via a
    K=3 fp32 PE matmul r = s*x + t - y and fused DVE/ACT count-accumulate;
    AllReduce of counts; on-device first-argmax -> (s,t); and the per-pixel
    maps 1/(d+1e-6), 1/max(s*d_pri+t, 1e-4) plus the masked-L1 partial sums,
    sharded over rows.
  * Host: final scalar assembly of the loss.
"""
import os
import numpy as np

H, W = 1024, 2048
NCORES = 8
ITERATIONS = 1000
SUB_N = 50000
THRESH = 0.01
LAMBDA_L1 = 0.5

PPC = SUB_N // NCORES          # points per core (6250)
CH = 512                       # point-chunk (PSUM bank / fp32 moving max)
NCH = (PPC + CH - 1) // CH     # 13
PPC_PAD = NCH * CH             # 6656
ROWS = H // NCORES             # 128 pixel rows per core
PIXCH = 1024
NPIX = W // PIXCH              # 2

X_PAD = np.float32(0.0)
Y_PAD = np.float32(1.5e38)
T_PAD = np.float32(-1.5e38)
IDX_PAD = np.float32(2.0e9)

def _chop(v, keep=10):
    b = v.view(np.uint32) & np.uint32(0xFFFFFFFF ^ ((1 << (23 - keep)) - 1))
    return b.view(np.float32)


def _split3(v):
    v = np.ascontiguousarray(v, np.float32)
    v0 = _chop(v)
    v1 = _chop((v - v0).astype(np.float32))
    v2 = _chop((v - v0 - v1).astype(np.float32))
    return v0, v1, v2


LAST_PROFILE = {}
LAST_DEBUG = {}
_PROGRAM_CACHE = {}


def _import_bass():
    import sys
    try:
        import concourse.bass  # noqa: F401
    except ImportError:
        for p in ("/opt/trn_rl_repo", "/root/.axon_site/_ro/trn_rl_repo"):
            if os.path.isdir(p) and p not in sys.path:
                sys.path.insert(0, p)
    import concourse.bass as bass
    import concourse.mybir as mybir
    import concourse.tile as tile
    import concourse.bacc as bacc
    return bass, mybir, tile, bacc


DVE_CHUNKS = frozenset(list(range(1, NCH, 2)) + [0])  # 7 of 13 on DVE

def _build_program(S, dyn, mm_dtype="f32r12", single_core=False):
    """Build the SPMD Bass program. S = padded survivor count (mult of 128)."""
    bass, mybir, tile, bacc = _import_bass()
    F32 = mybir.dt.float32
    BF16 = mybir.dt.bfloat16
    Alu = mybir.AluOpType
    Act = mybir.ActivationFunctionType
    AX = mybir.AxisListType

    NT = S // 128
    nc = bacc.Bacc("TRN2", target_bir_lowering=False, debug=False,
                   num_devices=1 if single_core else NCORES)

    # ---- I/O ----
    if mm_dtype == "f32r12":
        MMDT, KK = mybir.dt.float32r, 12
    else:
        MMDT, KK = F32, 3
    i_lhsT = nc.dram_tensor("lhsT", [KK, S], MMDT, kind="ExternalInput").ap()
    i_rhs = nc.dram_tensor("rhs3", [KK, PPC_PAD], MMDT, kind="ExternalInput").ap()
    i_idx = nc.dram_tensor("idxrow", [1, S], F32, kind="ExternalInput").ap()
    i_s = nc.dram_tensor("srow", [1, S], F32, kind="ExternalInput").ap()
    i_t = nc.dram_tensor("trow", [1, S], F32, kind="ExternalInput").ap()
    i_dren = nc.dram_tensor("dren", [ROWS, W], F32, kind="ExternalInput").ap()
    i_dpri = nc.dram_tensor("dpri", [ROWS, W], F32, kind="ExternalInput").ap()
    o_tinv = nc.dram_tensor("tinv", [ROWS, W], F32, kind="ExternalOutput").ap()
    o_met = nc.dram_tensor("metric", [ROWS, W], F32, kind="ExternalOutput").ap()
    o_l1p = nc.dram_tensor("l1p", [ROWS, 1], F32, kind="ExternalOutput").ap()
    o_cnt = nc.dram_tensor("cnts", [1, S], F32, kind="ExternalOutput").ap()

    cc_in = nc.dram_tensor("cc_in", [S], F32)
    cc_out = nc.dram_tensor("cc_out", [S], F32, addr_space="Shared")
    st_b = nc.dram_tensor("st_bounce", [1, 2], F32)

    with tile.TileContext(nc) as tc:
        with (
            tc.tile_pool(name="per", bufs=1) as per,      # persistent
            tc.tile_pool(name="scr", bufs=2) as scr,      # rotating scratch
            tc.tile_pool(name="ps", bufs=4, space="PSUM") as ps,
        ):
            # ---- persistent tiles / constants ----
            lhsT = per.tile([KK, S], MMDT)
            rhs = per.tile([KK, PPC_PAD], MMDT)
            nc.sync.dma_start(lhsT[:], i_lhsT[:])
            nc.sync.dma_start(rhs[:], i_rhs[:])

            dyn_tile = per.tile([128, CH], F32)
            nc.vector.memset(dyn_tile[:], dyn)
            dyn_col = per.tile([128, 1], F32)
            nc.vector.memset(dyn_col[:], dyn)
            eps_col = per.tile([128, 1], F32)
            zero_col = per.tile([128, 1], F32)

            dren = per.tile([ROWS, W], F32)
            dpri = per.tile([ROWS, W], F32)
            tinv = per.tile([ROWS, W], F32)
            met = per.tile([ROWS, W], F32)
            nc.sync.dma_start(dren[:], i_dren[:])
            nc.sync.dma_start(dpri[:], i_dpri[:])

            # ---- count phase ----
            cnt_cols = [per.tile([128, 2 * NCH], F32, tag=f"cntc{t}", name=f"cntc{t}") for t in range(NT)]
            sg_cols = [per.tile([128, 2 * NCH], F32, tag=f"sgc{t}", name=f"sgc{t}") for t in range(NT)]
            for t in range(NT):
                nc.vector.memset(cnt_cols[t][:], 0.0)
                nc.vector.memset(sg_cols[t][:], 0.0)

            n_dve_chunks = len([c for c in range(NCH) if c in DVE_CHUNKS])
            for ch in range(NCH):
                for t in range(NT):
                    r_ps = ps.tile([128, CH], F32, tag="r")
                    nc.tensor.matmul(r_ps[:], lhsT[:, bass.ts(t, 128)],
                                     rhs[:, bass.ts(ch, CH)])
                    if ch not in DVE_CHUNKS:
                        # ACT path: net counts via the sign trick
                        sg1 = scr.tile([128, CH], BF16, tag="sg")
                        nc.scalar.activation(
                            sg1[:], r_ps[:], Act.Sign, bias=dyn_col[:],
                            scale=-1.0,
                            accum_out=sg_cols[t][:, 2 * ch:2 * ch + 1])
                        sg2 = scr.tile([128, CH], BF16, tag="sg")
                        nc.scalar.activation(
                            sg2[:], r_ps[:], Act.Sign, bias=dyn_col[:],
                            scale=1.0,
                            accum_out=sg_cols[t][:, 2 * ch + 1:2 * ch + 2])
                    else:
                        # DVE path: two one-sided counts (A=#(r<dyn), B=#(r>-dyn))
                        cb1 = scr.tile([128, CH], BF16, tag="cb1")
                        nc.vector.tensor_scalar(
                            out=cb1[:], in0=r_ps[:], scalar1=dyn, scalar2=None,
                            op0=Alu.is_lt, op1=Alu.add,
                            accum_out=cnt_cols[t][:, 2 * ch:2 * ch + 1])
                        cb2 = scr.tile([128, CH], BF16, tag="cb2")
                        nc.vector.tensor_scalar(
                            out=cb2[:], in0=r_ps[:], scalar1=-dyn, scalar2=None,
                            op0=Alu.is_gt, op1=Alu.add,
                            accum_out=cnt_cols[t][:, 2 * ch + 1:2 * ch + 2])

            # Serialize ACT table sets: pixel Ln/Exp wait on the sign phase
            # via a real data dep (eps/zero cols derive from sg_cols).
            nc.vector.tensor_scalar(zero_col[:], sg_cols[0][:, 0:1], 0.0,
                                    None, op0=Alu.mult)
            nc.vector.tensor_scalar(eps_col[:], zero_col[:], 1e-6,
                                    None, op0=Alu.add)

            # per-tile counts = sum(cnt_cols) + floor(0.5*sum(sg_cols))
            for t in range(NT):
                c1 = scr.tile([128, 1], F32, tag="c1")
                nc.vector.reduce_sum(c1[:], cnt_cols[t][:], axis=AX.X)
                c2 = scr.tile([128, 1], F32, tag="c2")
                nc.vector.reduce_sum(c2[:], sg_cols[t][:], axis=AX.X)
                hf = scr.tile([128, 1], F32, tag="hf")
                nc.vector.tensor_scalar(hf[:], c2[:], 0.5, None, op0=Alu.mult)
                # floor(hf) for hf in {k, k+0.5}: RNE((hf-0.25)+2^23)-2^23
                md = scr.tile([128, 1], F32, tag="md")
                nc.vector.tensor_scalar(md[:], hf[:], 8388607.75, None, op0=Alu.add)
                fl = scr.tile([128, 1], F32, tag="fl")
                nc.vector.tensor_scalar(fl[:], md[:], -8388608.0, None, op0=Alu.add)
                c1b = scr.tile([128, 1], F32, tag="c1b")
                nc.vector.tensor_scalar(c1b[:], c1[:], float(-n_dve_chunks * CH),
                                        None, op0=Alu.add)
                ct = scr.tile([128, 1], F32, tag="ct")
                nc.vector.tensor_add(ct[:], c1b[:], fl[:])
                nc.sync.dma_start(
                    bass.AP(cc_in, t * 128, [[1, 128]]), ct[:])

            # ---- all-reduce counts across the 8 cores ----
            if single_core:
                nc.gpsimd.dma_start(cc_out[:], cc_in[:])
            else:
                nc.gpsimd.collective_compute(
                    "AllReduce", Alu.add,
                    replica_groups=[list(range(NCORES))],
                    ins=[cc_in[:]],
                    outs=[cc_out[:]],
                )

            # ---- argmax (first max, original iteration order) ----
            crow = per.tile([1, S], F32)
            nc.sync.dma_start(crow[:], bass.AP(cc_out, 0, [[0, 1], [1, S]]))
            nc.sync.dma_start(o_cnt[:], crow[:])
            irow = per.tile([1, S], F32)
            srow = per.tile([1, S], F32)
            trow = per.tile([1, S], F32)
            nc.sync.dma_start(irow[:], i_idx[:])
            nc.sync.dma_start(srow[:], i_s[:])
            nc.sync.dma_start(trow[:], i_t[:])

            mx = per.tile([1, 1], F32)
            nc.vector.reduce_max(mx[:], crow[:], axis=AX.X)
            nm = scr.tile([1, S], F32, tag="nm")
            nc.vector.tensor_scalar(nm[:], crow[:], mx[:, 0:1], 4.0e9,
                                    op0=Alu.not_equal, op1=Alu.mult)
            cand = scr.tile([1, S], F32, tag="cand")
            nc.vector.tensor_add(cand[:], nm[:], irow[:])
            best = per.tile([1, 1], F32)
            nc.vector.tensor_reduce(best[:], cand[:], axis=AX.X, op=Alu.min)

            oh1 = scr.tile([1, S], F32, tag="oh1")
            s_best = per.tile([1, 1], F32)
            nc.vector.scalar_tensor_tensor(
                out=oh1[:], in0=irow[:], scalar=best[:, 0:1], in1=srow[:],
                op0=Alu.is_equal, op1=Alu.mult, accum_out=s_best[:])
            oh2 = scr.tile([1, S], F32, tag="oh2")
            t_best = per.tile([1, 1], F32)
            nc.vector.scalar_tensor_tensor(
                out=oh2[:], in0=irow[:], scalar=best[:, 0:1], in1=trow[:],
                op0=Alu.is_equal, op1=Alu.mult, accum_out=t_best[:])

            # broadcast (s,t) to all 128 partitions via a DRAM bounce
            nc.sync.dma_start(bass.AP(st_b, 0, [[1, 1], [1, 1]]), s_best[:])
            nc.sync.dma_start(bass.AP(st_b, 1, [[1, 1], [1, 1]]), t_best[:])
            s_bc = per.tile([128, 1], F32)
            t_bc = per.tile([128, 1], F32)
            nc.sync.dma_start(s_bc[:], bass.AP(st_b, 0, [[0, 128], [1, 1]]))
            nc.sync.dma_start(t_bc[:], bass.AP(st_b, 1, [[0, 128], [1, 1]]))

            # ---- pixel phase ----
            # t_inv = exp(-ln(d_ren + 1e-6))   (independent of s,t)
            # phase 1: all Ln ops batched (one table set), aligned prep
    

            lndT = []
            lnmM = []
            alT = []
            for pc in range(NPIX):
                cs = bass.ts(pc, PIXCH)
                lnd = scr.tile([128, PIXCH], F32, tag=f"lndT{pc}", bufs=1,
                               name=f"lndT{pc}")
                nc.scalar.activation(lnd[:], dren[:, cs], Act.Ln,
                                     bias=eps_col[:], scale=1.0)
                lndT.append(lnd)
                v = scr.tile([128, PIXCH], F32, tag=f"alv{pc}", bufs=1,
                             name=f"alv{pc}")
                nc.vector.tensor_scalar(v[:], dpri[:, cs], s_bc[:, 0:1], None,
                                        op0=Alu.mult)
                nc.vector.tensor_scalar(v[:], v[:], t_bc[:, 0:1], None,
                                        op0=Alu.add)
                alT.append(v)
                mx2 = scr.tile([128, PIXCH], F32, tag="mx2")
                nc.vector.tensor_scalar(mx2[:], v[:], 1e-4, None, op0=Alu.max)
                lnm = scr.tile([128, PIXCH], F32, tag=f"lnmM{pc}", bufs=1,
                               name=f"lnmM{pc}")
                nc.scalar.activation(lnm[:], mx2[:], Act.Ln,
                                     bias=zero_col[:], scale=1.0)
                lnmM.append(lnm)
            # Exp ops wait for every Ln via a rebuilt bias column
            zeroE = per.tile([128, 1], F32)
            nc.vector.tensor_scalar(zeroE[:], lnmM[-1][:, 0:1], 0.0, None,
                                    op0=Alu.mult)

            l1cols = per.tile([128, NPIX], F32)
            for pc in range(NPIX):
                cs = bass.ts(pc, PIXCH)
                al = alT[pc]
                nc.scalar.activation(tinv[:, cs], lndT[pc][:], Act.Exp,
                                     bias=zeroE[:], scale=-1.0)
                nc.scalar.activation(met[:, cs], lnmM[pc][:], Act.Exp,
                                     bias=zeroE[:], scale=-1.0)
                # masked L1 partial: |al - tinv| * (dren>0.1) * (dren<100)
                dm = scr.tile([128, PIXCH], F32, tag="dm")
                nc.vector.tensor_sub(dm[:], al[:], tinv[:, cs])
                ab = dm
                I32 = mybir.dt.int32
                nc.vector.tensor_scalar(ab[:].bitcast(I32), dm[:].bitcast(I32),
                                        0x7FFFFFFF, None, op0=Alu.bitwise_and)
                m1 = scr.tile([128, PIXCH], F32, tag="m1")
                nc.vector.scalar_tensor_tensor(
                    out=m1[:], in0=dren[:, cs], scalar=0.1, in1=ab[:],
                    op0=Alu.is_gt, op1=Alu.mult)
                m2 = scr.tile([128, PIXCH], F32, tag="m2")
                nc.vector.scalar_tensor_tensor(
                    out=m2[:], in0=dren[:, cs], scalar=100.0, in1=m1[:],
                    op0=Alu.is_lt, op1=Alu.mult,
                    accum_out=l1cols[:, pc:pc + 1])
            nc.sync.dma_start(o_tinv[:], tinv[:])

            nc.sync.dma_start(o_met[:], met[:])
            l1p = per.tile([128, 1], F32)
            nc.vector.reduce_sum(l1p[:], l1cols[:], axis=AX.X)
            nc.sync.dma_start(o_l1p[:], l1p[:])

    nc.compile()
    return nc


def _host_control(rendered_depth, prior_disparity):
    """Exact host-side replication of the data-dependent control scalars."""
    import jax
    import jax.numpy as jnp
    cpu = jax.devices("cpu")[0]

    d_ren = np.asarray(rendered_depth, dtype=np.float32)
    d_pri = np.asarray(prior_disparity, dtype=np.float32)
    mask = (d_ren > 0.1) & (d_ren < 100.0) & np.isfinite(d_ren)
    mask_flat = mask.reshape(-1)
    num_pts = int(mask_flat.sum())
    n_valid = num_pts  # same expression in the reference
    P = d_ren.size

    x = d_pri.reshape(-1).astype(np.float32)
    y = (np.float32(1.0) / (d_ren.reshape(-1).astype(np.float32) + np.float32(1e-6)))

    idx_m = np.flatnonzero(mask_flat)
    if idx_m.size < P:
        idx_m = np.concatenate([idx_m, np.zeros(P - idx_m.size, idx_m.dtype)])

    with jax.default_device(cpu):
        k_pair, k_sub = jax.random.split(jax.random.key(42))
        pos = np.asarray(jax.random.randint(k_pair, (ITERATIONS, 2), 0, num_pts))
        spos = np.asarray(jax.random.randint(k_sub, (SUB_N,), 0, num_pts))
        y_nan = jnp.where(jnp.asarray(mask_flat), jnp.asarray(y), jnp.nan)
        med = jnp.nanmedian(y_nan)
        dynj = jnp.nanmedian(jnp.abs(y_nan - med)) * 0.5
        dynj = jnp.where(dynj < 1e-5, THRESH, dynj)
        dyn = np.float32(np.asarray(dynj))

    pi = idx_m[pos]
    x1, x2 = x[pi[:, 0]], x[pi[:, 1]]
    y1, y2 = y[pi[:, 0]], y[pi[:, 1]]
    scales = ((y2 - y1) / ((x2 - x1) + np.float32(1e-8))).astype(np.float32)
    shifts = (y1 - scales * x1).astype(np.float32)

    si = idx_m[spos]
    x_sub = x[si].astype(np.float32)
    y_sub = y[si].astype(np.float32)

    surv = np.flatnonzero(scales > 0)
    fallback = (num_pts < 10) or (surv.size == 0)
    if fallback:
        surv_s = np.ones(1, np.float32)
        surv_t = np.zeros(1, np.float32)
        surv_i = np.zeros(1, np.float32)
    else:
        surv_s = scales[surv].astype(np.float32)
        surv_t = shifts[surv].astype(np.float32)
        surv_i = surv.astype(np.float32)

    return dict(d_ren=d_ren, d_pri=d_pri, num_pts=num_pts, n_valid=n_valid,
                dyn=dyn, scales=scales, shifts=shifts, x_sub=x_sub,
                y_sub=y_sub, surv_s=surv_s, surv_t=surv_t, surv_i=surv_i,
                fallback=fallback)


def _prepare(rendered_depth, prior_disparity, mm_dtype="f32r"):
    """Host control + program build + per-core input maps."""
    hc = _host_control(rendered_depth, prior_disparity)
    return _make_inputs(hc, mm_dtype) + (hc,)


def _make_inputs(hc, mm_dtype="f32r12"):
    dyn = float(hc["dyn"])

    # pad survivors to a multiple of 128
    S_real = hc["surv_s"].size
    NT = max(1, (S_real + 127) // 128)
    S = NT * 128
    s_full = np.ones(S, np.float32)
    t_full = np.full(S, T_PAD, np.float32)
    s_full[:S_real] = hc["surv_s"]
    t_full[:S_real] = hc["surv_t"]
    if mm_dtype == "f32r12":
        s0, s1, s2 = _split3(s_full)
        t0, t1, t2 = _split3(t_full)
        mone = -np.ones(S, np.float32)
        lhsT = np.stack([s0, s0, s1, s1, s0, s2, t0, t1, t2, mone, mone, mone])
    else:
        lhsT = np.stack([s_full, t_full, -np.ones(S, np.float32)])
    idxrow = np.full((1, S), IDX_PAD, np.float32)
    idxrow[0, :S_real] = hc["surv_i"]
    srow = np.ones((1, S), np.float32)
    srow[0, :S_real] = hc["surv_s"]
    trow = np.zeros((1, S), np.float32)
    trow[0, :S_real] = hc["surv_t"]

    key = (S, float(dyn), mm_dtype)
    if key not in _PROGRAM_CACHE:
        _PROGRAM_CACHE[key] = _build_program(S, dyn, mm_dtype)
    nc = _PROGRAM_CACHE[key]

    # per-core inputs
    in_maps = []
    for c in range(NCORES):
        xs = np.full(PPC_PAD, X_PAD, np.float32)
        ys = np.full(PPC_PAD, Y_PAD, np.float32)
        xs[:PPC] = hc["x_sub"][c * PPC:(c + 1) * PPC]
        ys[:PPC] = hc["y_sub"][c * PPC:(c + 1) * PPC]
        one = np.ones(PPC_PAD, np.float32)
        if mm_dtype == "f32r12":
            x0, x1, x2 = _split3(xs)
            y0, y1, y2 = _split3(ys)
            rhs3 = np.stack([x0, x1, x0, x1, x2, x0, one, one, one, y0, y1, y2])
        else:
            rhs3 = np.stack([xs, one, ys])
        rs = slice(c * ROWS, (c + 1) * ROWS)
        in_maps.append({
            "lhsT": lhsT, "rhs3": rhs3, "idxrow": idxrow,
            "srow": srow, "trow": trow,
            "dren": np.ascontiguousarray(hc["d_ren"][rs]),
            "dpri": np.ascontiguousarray(hc["d_pri"][rs]),
        })
    return nc, in_maps


def _finalize(results, hc):
    t_inv = np.concatenate([results[c]["tinv"] for c in range(NCORES)], axis=0)
    metric = np.concatenate([results[c]["metric"] for c in range(NCORES)], axis=0)
    l1_sum = float(sum(results[c]["l1p"].astype(np.float64).sum()
                       for c in range(NCORES)))
    n_valid = hc["n_valid"]
    l1 = l1_sum / max(n_valid, 1)
    total = np.float32(LAMBDA_L1 * l1)
    if n_valid < 100:
        total = np.float32(0.0)
    LAST_DEBUG.clear()
    LAST_DEBUG["cnts"] = results[0]["cnts"][0]
    LAST_DEBUG["hc"] = hc
    return total, t_inv, metric


def _exact_argmax_host(hc):
    """Reference-exact RANSAC winner, computed on host (fallback path)."""
    scales, shifts = hc["scales"], hc["shifts"]
    x_sub, y_sub, dyn = hc["x_sub"], hc["y_sub"], np.float32(hc["dyn"])
    counts = np.zeros(ITERATIONS, np.int64)
    CHh = 64
    for i0 in range(0, ITERATIONS, CHh):
        ss = scales[i0:i0 + CHh, None]
        tt = shifts[i0:i0 + CHh, None]
        w = ((ss * x_sub[None, :]).astype(np.float32) + tt).astype(np.float32)
        res = np.abs((w - y_sub[None, :]).astype(np.float32))
        counts[i0:i0 + CHh] = (res < dyn).sum(axis=1)
    cm = np.where(scales > 0, counts, -1)
    best = int(np.argmax(cm))
    if cm[best] >= 0 and hc["num_pts"] >= 10:
        return np.float32(scales[best]), np.float32(shifts[best])
    return np.float32(1.0), np.float32(0.0)


MARGIN_MIN = 64.0


def kernel(rendered_depth, prior_disparity):
    _import_bass()
    from concourse.bass_utils import run_bass_kernel_spmd

    hc = _host_control(rendered_depth, prior_disparity)
    nc, in_maps = _make_inputs(hc, "f32r12")
    trace = bool(os.environ.get("DEPTH_KERNEL_TRACE"))
    if trace:
        try:
            from antenv.axon_hooks import get_axon_ntff_profile_hook  # noqa: F401
        except ImportError:
            trace = False
    res = run_bass_kernel_spmd(nc, in_maps, list(range(NCORES)), trace=trace)
    LAST_PROFILE.clear()
    LAST_PROFILE["exec_time_ns"] = res.exec_time_ns
    LAST_PROFILE["res"] = res

    # Robustness certificate: the fp32r count matrix can deviate from the
    # reference-exact counts by a few units per iteration near the threshold.
    # If the observed top-2 margin is not comfortably larger than that, fall
    # back to the host-exact winner and rerun the (cheap) pixel phase with a
    # single forced candidate.
    S_real = int(hc["surv_s"].size)
    if not hc["fallback"] and S_real > 1:
        cnts = np.sort(res.results[0]["cnts"][0, :S_real])[::-1]
        margin_ok = bool(np.isfinite(cnts).all()) and \
            float(cnts[0] - cnts[1]) >= MARGIN_MIN
        if not margin_ok:
            s_b, t_b = _exact_argmax_host(hc)
            hc = dict(hc)
            hc["surv_s"] = np.array([s_b], np.float32)
            hc["surv_t"] = np.array([t_b], np.float32)
            hc["surv_i"] = np.zeros(1, np.float32)
            nc, in_maps = _make_inputs(hc, "f32r12")
            res = run_bass_kernel_spmd(nc, in_maps, list(range(NCORES)),
                                       trace=trace)
    return _finalize(res.results, hc)


# revision 26
# speedup vs baseline: 1.0437x; 1.0267x over previous
"""Trainium2 Bass kernel for nn_DepthPriorLoss (RANSAC depth-prior alignment).

Contract: kernel(**inputs) takes the FULL inputs from setup_inputs() and
returns the FULL outputs of reference():
    (total_loss, target_inv_ren [1024,2048], prior_metric_depth [1024,2048])

Strategy (8 NeuronCores, SPMD):
  * Host (exact, cheap): mask/num_pts, RANSAC random pairs (jax CPU threefry,
    bit-exact with the reference), scales/shifts, the MAD threshold `dyn`
    (jnp.nanmedian on CPU, bit-exact), survivor (s>0) pruning.
  * Device: the O(50M) inlier-count matrix, sharded over points (each core
    evaluates all surviving (s,t) candidates against SUB_N/8 points), via a
    K=3 fp32 PE matmul r = s*x + t - y and fused DVE/ACT count-accumulate;
    AllReduce of counts; on-device first-argmax -> (s,t); and the per-pixel
    maps 1/(d+1e-6), 1/max(s*d_pri+t, 1e-4) plus the masked-L1 partial sums,
    sharded over rows.
  * Host: final scalar assembly of the loss.
"""
import os
import numpy as np

H, W = 1024, 2048
NCORES = 8
ITERATIONS = 1000
SUB_N = 50000
THRESH = 0.01
LAMBDA_L1 = 0.5

PPC = SUB_N // NCORES          # points per core (6250)
CH = 512                       # point-chunk (PSUM bank / fp32 moving max)
NCH = (PPC + CH - 1) // CH     # 13
PPC_PAD = NCH * CH             # 6656
ROWS = H // NCORES             # 128 pixel rows per core
PIXCH = 1024
NPIX = W // PIXCH              # 2

X_PAD = np.float32(0.0)
Y_PAD = np.float32(1.5e38)
T_PAD = np.float32(-1.5e38)
IDX_PAD = np.float32(2.0e9)

def _chop(v, keep=10):
    b = v.view(np.uint32) & np.uint32(0xFFFFFFFF ^ ((1 << (23 - keep)) - 1))
    return b.view(np.float32)


def _split3(v):
    v = np.ascontiguousarray(v, np.float32)
    v0 = _chop(v)
    v1 = _chop((v - v0).astype(np.float32))
    v2 = _chop((v - v0 - v1).astype(np.float32))
    return v0, v1, v2


LAST_PROFILE = {}
LAST_DEBUG = {}
_PROGRAM_CACHE = {}


def _import_bass():
    import sys
    try:
        import concourse.bass  # noqa: F401
    except ImportError:
        for p in ("/opt/trn_rl_repo", "/root/.axon_site/_ro/trn_rl_repo"):
            if os.path.isdir(p) and p not in sys.path:
                sys.path.insert(0, p)
    import concourse.bass as bass
    import concourse.mybir as mybir
    import concourse.tile as tile
    import concourse.bacc as bacc
    return bass, mybir, tile, bacc


DVE_CHUNKS = frozenset(list(range(1, NCH, 2)) + [0])  # 7 of 13 on DVE

def _build_program(S, dyn, mm_dtype="f32r12", single_core=False):
    """Build the SPMD Bass program. S = padded survivor count (mult of 128)."""
    bass, mybir, tile, bacc = _import_bass()
    F32 = mybir.dt.float32
    BF16 = mybir.dt.bfloat16
    Alu = mybir.AluOpType
    Act = mybir.ActivationFunctionType
    AX = mybir.AxisListType

    NT = S // 128
    nc = bacc.Bacc("TRN2", target_bir_lowering=False, debug=False,
                   num_devices=1 if single_core else NCORES)

    # ---- I/O ----
    if mm_dtype == "f32r12":
        MMDT, KK = mybir.dt.float32r, 12
    else:
        MMDT, KK = F32, 3
    i_lhsT = nc.dram_tensor("lhsT", [KK, S], MMDT, kind="ExternalInput").ap()
    i_rhs = nc.dram_tensor("rhs3", [KK, PPC_PAD], MMDT, kind="ExternalInput").ap()
    i_idx = nc.dram_tensor("idxrow", [1, S], F32, kind="ExternalInput").ap()
    i_s = nc.dram_tensor("srow", [1, S], F32, kind="ExternalInput").ap()
    i_t = nc.dram_tensor("trow", [1, S], F32, kind="ExternalInput").ap()
    i_dren = nc.dram_tensor("dren", [ROWS, W], F32, kind="ExternalInput").ap()
    i_dpri = nc.dram_tensor("dpri", [ROWS, W], F32, kind="ExternalInput").ap()
    o_tinv = nc.dram_tensor("tinv", [ROWS, W], F32, kind="ExternalOutput").ap()
    o_met = nc.dram_tensor("metric", [ROWS, W], F32, kind="ExternalOutput").ap()
    o_l1p = nc.dram_tensor("l1p", [ROWS, 1], F32, kind="ExternalOutput").ap()
    o_cnt = nc.dram_tensor("cnts", [1, S], F32, kind="ExternalOutput").ap()

    cc_in = nc.dram_tensor("cc_in", [S], F32)
    cc_out = nc.dram_tensor("cc_out", [S], F32, addr_space="Shared")
    st_b = nc.dram_tensor("st_bounce", [1, 2], F32)

    with tile.TileContext(nc) as tc:
        with (
            tc.tile_pool(name="per", bufs=1) as per,      # persistent
            tc.tile_pool(name="scr", bufs=2) as scr,      # rotating scratch
            tc.tile_pool(name="ps", bufs=3, space="PSUM") as ps,
        ):
            # ---- persistent tiles / constants ----
            lhsT = per.tile([KK, S], MMDT)
            rhs = per.tile([KK, PPC_PAD], MMDT)
            nc.sync.dma_start(lhsT[:], i_lhsT[:])
            nc.sync.dma_start(rhs[:], i_rhs[:])

            dyn_tile = per.tile([128, CH], F32)
            nc.vector.memset(dyn_tile[:], dyn)
            dyn_col = per.tile([128, 1], F32)
            nc.vector.memset(dyn_col[:], dyn)
            eps_col = per.tile([128, 1], F32)
            zero_col = per.tile([128, 1], F32)

            dren = per.tile([ROWS, W], F32)
            dpri = per.tile([ROWS, W], F32)
            tinv = per.tile([ROWS, W], F32)
            met = per.tile([ROWS, W], F32)
            nc.sync.dma_start(dren[:], i_dren[:])
            nc.sync.dma_start(dpri[:], i_dpri[:])

            # ---- count phase ----
            W2 = 2 * NCH
            cnt_all = per.tile([128, NT * W2], F32)
            sg_all = per.tile([128, NT * W2], F32)
            nc.vector.memset(cnt_all[:], 0.0)
            nc.vector.memset(sg_all[:], 0.0)
            cnt_cols = [cnt_all[:, t * W2:(t + 1) * W2] for t in range(NT)]
            sg_cols = [sg_all[:, t * W2:(t + 1) * W2] for t in range(NT)]

            # pair point-chunks: two PSUM banks per consumer op (wider ops
            # amortize the DVE/ACT per-instruction overhead)
    

            pairs = []
            ch = 0
            while ch < NCH:
                w = 2 * CH if ch + 1 < NCH else CH
                pairs.append((ch, w))
                ch += w // CH
            # DVE handles ~54% of the pairs, ACT the rest
            dve_pairs = {0, 2, 4, 6}
            n_dve_chunks = sum(pairs[p][1] // CH for p in dve_pairs)
            for pi, (ch, w) in enumerate(pairs):
                for t in range(NT):
                    r_ps = ps.tile([128, w], F32, tag="r")
                    nc.tensor.matmul(r_ps[:, 0:CH], lhsT[:, bass.ts(t, 128)],
                                     rhs[:, bass.ts(ch, CH)])
                    if w == 2 * CH:
                        nc.tensor.matmul(r_ps[:, CH:2 * CH],
                                         lhsT[:, bass.ts(t, 128)],
                                         rhs[:, bass.ts(ch + 1, CH)])
                    if pi not in dve_pairs:
                        # ACT path: net counts via the sign trick
                        sg1 = scr.tile([128, w], BF16, tag="sg")
                        nc.scalar.activation(
                            sg1[:], r_ps[:], Act.Sign, bias=dyn_col[:],
                            scale=-1.0,
                            accum_out=sg_cols[t][:, 2 * pi:2 * pi + 1])
                        sg2 = scr.tile([128, w], BF16, tag="sg")
                        nc.scalar.activation(
                            sg2[:], r_ps[:], Act.Sign, bias=dyn_col[:],
                            scale=1.0,
                            accum_out=sg_cols[t][:, 2 * pi + 1:2 * pi + 2])
                    else:
                        # DVE path: two one-sided counts (A=#(r<dyn), B=#(r>-dyn))
                        cb1 = scr.tile([128, w], BF16, tag="cb1")
                        nc.vector.tensor_scalar(
                            out=cb1[:], in0=r_ps[:], scalar1=dyn, scalar2=None,
                            op0=Alu.is_lt, op1=Alu.add,
                            accum_out=cnt_cols[t][:, 2 * pi:2 * pi + 1])
                        cb2 = scr.tile([128, w], BF16, tag="cb2")
                        nc.vector.tensor_scalar(
                            out=cb2[:], in0=r_ps[:], scalar1=-dyn, scalar2=None,
                            op0=Alu.is_gt, op1=Alu.add,
                            accum_out=cnt_cols[t][:, 2 * pi + 1:2 * pi + 2])

            # Serialize ACT table sets: pixel Ln/Exp wait on the sign phase
            # via a real data dep (eps/zero cols derive from sg_cols).
            nc.vector.tensor_scalar(zero_col[:], sg_all[:, 0:1], 0.0,
                                    None, op0=Alu.mult)
            nc.vector.tensor_scalar(eps_col[:], zero_col[:], 1e-6,
                                    None, op0=Alu.add)

            # counts = sum(cnt cols) + floor(0.5*sum(sg cols)), all NT tiles
            # vectorized as [128, NT] ops
            c1 = scr.tile([128, NT], F32, tag="c1")
            nc.vector.reduce_sum(
                c1[:], cnt_all[:].rearrange("p (t c) -> p t c", c=W2),
                axis=AX.X)
            c2 = scr.tile([128, NT], F32, tag="c2")
            nc.vector.reduce_sum(
                c2[:], sg_all[:].rearrange("p (t c) -> p t c", c=W2),
                axis=AX.X)
            hf = scr.tile([128, NT], F32, tag="hf")
            nc.vector.tensor_scalar(hf[:], c2[:], 0.5, None, op0=Alu.mult)
            # floor(hf) for hf in {k, k+0.5}: RNE((hf-0.25)+2^23)-2^23
            nc.vector.tensor_scalar(hf[:], hf[:], 8388607.75, None, op0=Alu.add)
            nc.vector.tensor_scalar(hf[:], hf[:], -8388608.0, None, op0=Alu.add)
            nc.vector.tensor_scalar(c1[:], c1[:], float(-n_dve_chunks * CH),
                                    None, op0=Alu.add)
            ct = scr.tile([128, NT], F32, tag="ct")
            nc.vector.tensor_add(ct[:], c1[:], hf[:])
            for t in range(NT):
                nc.sync.dma_start(
                    bass.AP(cc_in, t * 128, [[1, 128]]), ct[:, t:t + 1])

            # ---- all-reduce counts across the 8 cores ----
            if single_core:
                nc.gpsimd.dma_start(cc_out[:], cc_in[:])
            else:
                nc.gpsimd.collective_compute(
                    "AllReduce", Alu.add,
                    replica_groups=[list(range(NCORES))],
                    ins=[cc_in[:]],
                    outs=[cc_out[:]],
                )

            # ---- argmax (first max, original iteration order) ----
            crow = per.tile([1, S], F32)
            nc.sync.dma_start(crow[:], bass.AP(cc_out, 0, [[0, 1], [1, S]]))
            nc.sync.dma_start(o_cnt[:], crow[:])
            irow = per.tile([1, S], F32)
            srow = per.tile([1, S], F32)
            trow = per.tile([1, S], F32)
            nc.sync.dma_start(irow[:], i_idx[:])
            nc.sync.dma_start(srow[:], i_s[:])
            nc.sync.dma_start(trow[:], i_t[:])

            mx = per.tile([1, 1], F32)
            nc.vector.reduce_max(mx[:], crow[:], axis=AX.X)
            nm = scr.tile([1, S], F32, tag="nm")
            nc.vector.tensor_scalar(nm[:], crow[:], mx[:, 0:1], 4.0e9,
                                    op0=Alu.not_equal, op1=Alu.mult)
            cand = scr.tile([1, S], F32, tag="cand")
            nc.vector.tensor_add(cand[:], nm[:], irow[:])
            best = per.tile([1, 1], F32)
            nc.vector.tensor_reduce(best[:], cand[:], axis=AX.X, op=Alu.min)

            oh1 = scr.tile([1, S], F32, tag="oh1")
            s_best = per.tile([1, 1], F32)
            nc.vector.scalar_tensor_tensor(
                out=oh1[:], in0=irow[:], scalar=best[:, 0:1], in1=srow[:],
                op0=Alu.is_equal, op1=Alu.mult, accum_out=s_best[:])
            oh2 = scr.tile([1, S], F32, tag="oh2")
            t_best = per.tile([1, 1], F32)
            nc.vector.scalar_tensor_tensor(
                out=oh2[:], in0=irow[:], scalar=best[:, 0:1], in1=trow[:],
                op0=Alu.is_equal, op1=Alu.mult, accum_out=t_best[:])

            # broadcast (s,t) to all 128 partitions via a DRAM bounce
            nc.sync.dma_start(bass.AP(st_b, 0, [[1, 1], [1, 1]]), s_best[:])
            nc.sync.dma_start(bass.AP(st_b, 1, [[1, 1], [1, 1]]), t_best[:])
            s_bc = per.tile([128, 1], F32)
            t_bc = per.tile([128, 1], F32)
            nc.sync.dma_start(s_bc[:], bass.AP(st_b, 0, [[0, 128], [1, 1]]))
            nc.sync.dma_start(t_bc[:], bass.AP(st_b, 1, [[0, 128], [1, 1]]))

            # ---- pixel phase ----
            # t_inv = exp(-ln(d_ren + 1e-6))   (independent of s,t)
            # phase 1: all Ln ops batched (one table set), aligned prep
    

            lndT = []
            lnmM = []
            alT = []
            for pc in range(NPIX):
                cs = bass.ts(pc, PIXCH)
                lnd = scr.tile([128, PIXCH], F32, tag=f"lndT{pc}", bufs=1,
                               name=f"lndT{pc}")
                nc.scalar.activation(lnd[:], dren[:, cs], Act.Ln,
                                     bias=eps_col[:], scale=1.0)
                lndT.append(lnd)
                v = scr.tile([128, PIXCH], F32, tag=f"alv{pc}", bufs=1,
                             name=f"alv{pc}")
                nc.vector.tensor_scalar(v[:], dpri[:, cs], s_bc[:, 0:1], None,
                                        op0=Alu.mult)
                nc.vector.tensor_scalar(v[:], v[:], t_bc[:, 0:1], None,
                                        op0=Alu.add)
                alT.append(v)
                mx2 = scr.tile([128, PIXCH], F32, tag="mx2")
                nc.vector.tensor_scalar(mx2[:], v[:], 1e-4, None, op0=Alu.max)
                lnm = scr.tile([128, PIXCH], F32, tag=f"lnmM{pc}", bufs=1,
                               name=f"lnmM{pc}")
                nc.scalar.activation(lnm[:], mx2[:], Act.Ln,
                                     bias=zero_col[:], scale=1.0)
                lnmM.append(lnm)
            # Exp ops wait for every Ln via a rebuilt bias column
            zeroE = per.tile([128, 1], F32)
            nc.vector.tensor_scalar(zeroE[:], lnmM[-1][:, 0:1], 0.0, None,
                                    op0=Alu.mult)

            l1cols = per.tile([128, NPIX], F32)
            for pc in range(NPIX):
                cs = bass.ts(pc, PIXCH)
                al = alT[pc]
                nc.scalar.activation(tinv[:, cs], lndT[pc][:], Act.Exp,
                                     bias=zeroE[:], scale=-1.0)
                nc.scalar.activation(met[:, cs], lnmM[pc][:], Act.Exp,
                                     bias=zeroE[:], scale=-1.0)
                # masked L1 partial: |al - tinv| * (dren>0.1) * (dren<100)
                dm = scr.tile([128, PIXCH], F32, tag="dm")
                nc.gpsimd.tensor_tensor(dm[:], al[:], tinv[:, cs],
                                        op=Alu.subtract)
                ab = dm
                I32 = mybir.dt.int32
                nc.vector.tensor_scalar(ab[:].bitcast(I32), dm[:].bitcast(I32),
                                        0x7FFFFFFF, None, op0=Alu.bitwise_and)
                m1 = scr.tile([128, PIXCH], F32, tag="m1")
                nc.vector.scalar_tensor_tensor(
                    out=m1[:], in0=dren[:, cs], scalar=0.1, in1=ab[:],
                    op0=Alu.is_gt, op1=Alu.mult)
                m2 = scr.tile([128, PIXCH], F32, tag="m2")
                nc.vector.scalar_tensor_tensor(
                    out=m2[:], in0=dren[:, cs], scalar=100.0, in1=m1[:],
                    op0=Alu.is_lt, op1=Alu.mult,
                    accum_out=l1cols[:, pc:pc + 1])
            nc.sync.dma_start(o_tinv[:], tinv[:])

            nc.sync.dma_start(o_met[:], met[:])
            l1p = per.tile([128, 1], F32)
            nc.vector.reduce_sum(l1p[:], l1cols[:], axis=AX.X)
            nc.sync.dma_start(o_l1p[:], l1p[:])

    nc.compile()
    return nc


def _host_control(rendered_depth, prior_disparity):
    """Exact host-side replication of the data-dependent control scalars."""
    import jax
    import jax.numpy as jnp
    cpu = jax.devices("cpu")[0]

    d_ren = np.asarray(rendered_depth, dtype=np.float32)
    d_pri = np.asarray(prior_disparity, dtype=np.float32)
    mask = (d_ren > 0.1) & (d_ren < 100.0) & np.isfinite(d_ren)
    mask_flat = mask.reshape(-1)
    num_pts = int(mask_flat.sum())
    n_valid = num_pts  # same expression in the reference
    P = d_ren.size

    x = d_pri.reshape(-1).astype(np.float32)
    y = (np.float32(1.0) / (d_ren.reshape(-1).astype(np.float32) + np.float32(1e-6)))

    idx_m = np.flatnonzero(mask_flat)
    if idx_m.size < P:
        idx_m = np.concatenate([idx_m, np.zeros(P - idx_m.size, idx_m.dtype)])

    with jax.default_device(cpu):
        k_pair, k_sub = jax.random.split(jax.random.key(42))
        pos = np.asarray(jax.random.randint(k_pair, (ITERATIONS, 2), 0, num_pts))
        spos = np.asarray(jax.random.randint(k_sub, (SUB_N,), 0, num_pts))
        y_nan = jnp.where(jnp.asarray(mask_flat), jnp.asarray(y), jnp.nan)
        med = jnp.nanmedian(y_nan)
        dynj = jnp.nanmedian(jnp.abs(y_nan - med)) * 0.5
        dynj = jnp.where(dynj < 1e-5, THRESH, dynj)
        dyn = np.float32(np.asarray(dynj))

    pi = idx_m[pos]
    x1, x2 = x[pi[:, 0]], x[pi[:, 1]]
    y1, y2 = y[pi[:, 0]], y[pi[:, 1]]
    scales = ((y2 - y1) / ((x2 - x1) + np.float32(1e-8))).astype(np.float32)
    shifts = (y1 - scales * x1).astype(np.float32)

    si = idx_m[spos]
    x_sub = x[si].astype(np.float32)
    y_sub = y[si].astype(np.float32)

    surv = np.flatnonzero(scales > 0)
    fallback = (num_pts < 10) or (surv.size == 0)
    if fallback:
        surv_s = np.ones(1, np.float32)
        surv_t = np.zeros(1, np.float32)
        surv_i = np.zeros(1, np.float32)
    else:
        surv_s = scales[surv].astype(np.float32)
        surv_t = shifts[surv].astype(np.float32)
        surv_i = surv.astype(np.float32)

    return dict(d_ren=d_ren, d_pri=d_pri, num_pts=num_pts, n_valid=n_valid,
                dyn=dyn, scales=scales, shifts=shifts, x_sub=x_sub,
                y_sub=y_sub, surv_s=surv_s, surv_t=surv_t, surv_i=surv_i,
                fallback=fallback)


def _prepare(rendered_depth, prior_disparity, mm_dtype="f32r"):
    """Host control + program build + per-core input maps."""
    hc = _host_control(rendered_depth, prior_disparity)
    return _make_inputs(hc, mm_dtype) + (hc,)


def _make_inputs(hc, mm_dtype="f32r12"):
    dyn = float(hc["dyn"])

    # pad survivors to a multiple of 128
    S_real = hc["surv_s"].size
    NT = max(1, (S_real + 127) // 128)
    S = NT * 128
    s_full = np.ones(S, np.float32)
    t_full = np.full(S, T_PAD, np.float32)
    s_full[:S_real] = hc["surv_s"]
    t_full[:S_real] = hc["surv_t"]
    if mm_dtype == "f32r12":
        s0, s1, s2 = _split3(s_full)
        t0, t1, t2 = _split3(t_full)
        mone = -np.ones(S, np.float32)
        lhsT = np.stack([s0, s0, s1, s1, s0, s2, t0, t1, t2, mone, mone, mone])
    else:
        lhsT = np.stack([s_full, t_full, -np.ones(S, np.float32)])
    idxrow = np.full((1, S), IDX_PAD, np.float32)
    idxrow[0, :S_real] = hc["surv_i"]
    srow = np.ones((1, S), np.float32)
    srow[0, :S_real] = hc["surv_s"]
    trow = np.zeros((1, S), np.float32)
    trow[0, :S_real] = hc["surv_t"]

    key = (S, float(dyn), mm_dtype)
    if key not in _PROGRAM_CACHE:
        _PROGRAM_CACHE[key] = _build_program(S, dyn, mm_dtype)
    nc = _PROGRAM_CACHE[key]

    # per-core inputs
    in_maps = []
    for c in range(NCORES):
        xs = np.full(PPC_PAD, X_PAD, np.float32)
        ys = np.full(PPC_PAD, Y_PAD, np.float32)
        xs[:PPC] = hc["x_sub"][c * PPC:(c + 1) * PPC]
        ys[:PPC] = hc["y_sub"][c * PPC:(c + 1) * PPC]
        one = np.ones(PPC_PAD, np.float32)
        if mm_dtype == "f32r12":
            x0, x1, x2 = _split3(xs)
            y0, y1, y2 = _split3(ys)
            rhs3 = np.stack([x0, x1, x0, x1, x2, x0, one, one, one, y0, y1, y2])
        else:
            rhs3 = np.stack([xs, one, ys])
        rs = slice(c * ROWS, (c + 1) * ROWS)
        in_maps.append({
            "lhsT": lhsT, "rhs3": rhs3, "idxrow": idxrow,
            "srow": srow, "trow": trow,
            "dren": np.ascontiguousarray(hc["d_ren"][rs]),
            "dpri": np.ascontiguousarray(hc["d_pri"][rs]),
        })
    return nc, in_maps


def _finalize(results, hc):
    t_inv = np.concatenate([results[c]["tinv"] for c in range(NCORES)], axis=0)
    metric = np.concatenate([results[c]["metric"] for c in range(NCORES)], axis=0)
    l1_sum = float(sum(results[c]["l1p"].astype(np.float64).sum()
                       for c in range(NCORES)))
    n_valid = hc["n_valid"]
    l1 = l1_sum / max(n_valid, 1)
    total = np.float32(LAMBDA_L1 * l1)
    if n_valid < 100:
        total = np.float32(0.0)
    LAST_DEBUG.clear()
    LAST_DEBUG["cnts"] = results[0]["cnts"][0]
    LAST_DEBUG["hc"] = hc
    return total, t_inv, metric


def _exact_argmax_host(hc):
    """Reference-exact RANSAC winner, computed on host (fallback path)."""
    scales, shifts = hc["scales"], hc["shifts"]
    x_sub, y_sub, dyn = hc["x_sub"], hc["y_sub"], np.float32(hc["dyn"])
    counts = np.zeros(ITERATIONS, np.int64)
    CHh = 64
    for i0 in range(0, ITERATIONS, CHh):
        ss = scales[i0:i0 + CHh, None]
        tt = shifts[i0:i0 + CHh, None]
        w = ((ss * x_sub[None, :]).astype(np.float32) + tt).astype(np.float32)
        res = np.abs((w - y_sub[None, :]).astype(np.float32))
        counts[i0:i0 + CHh] = (res < dyn).sum(axis=1)
    cm = np.where(scales > 0, counts, -1)
    best = int(np.argmax(cm))
    if cm[best] >= 0 and hc["num_pts"] >= 10:
        return np.float32(scales[best]), np.float32(shifts[best])
    return np.float32(1.0), np.float32(0.0)


MARGIN_MIN = 64.0


def kernel(rendered_depth, prior_disparity):
    _import_bass()
    from concourse.bass_utils import run_bass_kernel_spmd

    hc = _host_control(rendered_depth, prior_disparity)
    nc, in_maps = _make_inputs(hc, "f32r12")
    trace = bool(os.environ.get("DEPTH_KERNEL_TRACE"))
    if trace:
        try:
            from antenv.axon_hooks import get_axon_ntff_profile_hook  # noqa: F401
        except ImportError:
            trace = False
    res = run_bass_kernel_spmd(nc, in_maps, list(range(NCORES)), trace=trace)
    LAST_PROFILE.clear()
    LAST_PROFILE["exec_time_ns"] = res.exec_time_ns
    LAST_PROFILE["res"] = res

    # Robustness certificate: the fp32r count matrix can deviate from the
    # reference-exact counts by a few units per iteration near the threshold.
    # If the observed top-2 margin is not comfortably larger than that, fall
    # back to the host-exact winner and rerun the (cheap) pixel phase with a
    # single forced candidate.
    S_real = int(hc["surv_s"].size)
    if not hc["fallback"] and S_real > 1:
        cnts = np.sort(res.results[0]["cnts"][0, :S_real])[::-1]
        margin_ok = bool(np.isfinite(cnts).all()) and \
            float(cnts[0] - cnts[1]) >= MARGIN_MIN
        if not margin_ok:
            s_b, t_b = _exact_argmax_host(hc)
            hc = dict(hc)
            hc["surv_s"] = np.array([s_b], np.float32)
            hc["surv_t"] = np.array([t_b], np.float32)
            hc["surv_i"] = np.zeros(1, np.float32)
            nc, in_maps = _make_inputs(hc, "f32r12")
            res = run_bass_kernel_spmd(nc, in_maps, list(range(NCORES)),
                                       trace=trace)
    return _finalize(res.results, hc)


# revision 28
# speedup vs baseline: 1.0955x; 1.0496x over previous
"""Trainium2 Bass kernel for nn_DepthPriorLoss (RANSAC depth-prior alignment).

Contract: kernel(**inputs) takes the FULL inputs from setup_inputs() and
returns the FULL outputs of reference():
    (total_loss, target_inv_ren [1024,2048], prior_metric_depth [1024,2048])

Strategy (8 NeuronCores, SPMD):
  * Host (exact, cheap): mask/num_pts, RANSAC random pairs (jax CPU threefry,
    bit-exact with the reference), scales/shifts, the MAD threshold `dyn`
    (jnp.nanmedian on CPU, bit-exact), survivor (s>0) pruning.
  * Device: the O(50M) inlier-count matrix, sharded over points (each core
    evaluates all surviving (s,t) candidates against SUB_N/8 points), via a
    K=3 fp32 PE matmul r = s*x + t - y and fused DVE/ACT count-accumulate;
    AllReduce of counts; on-device first-argmax -> (s,t); and the per-pixel
    maps 1/(d+1e-6), 1/max(s*d_pri+t, 1e-4) plus the masked-L1 partial sums,
    sharded over rows.
  * Host: final scalar assembly of the loss.
"""
import os
import numpy as np

H, W = 1024, 2048
NCORES = 8
ITERATIONS = 1000
SUB_N = 50000
THRESH = 0.01
LAMBDA_L1 = 0.5

PPC = SUB_N // NCORES          # points per core (6250)
CH = 512                       # point-chunk (PSUM bank / fp32 moving max)
NCH = (PPC + CH - 1) // CH     # 13
PPC_PAD = NCH * CH             # 6656
ROWS = H // NCORES             # 128 pixel rows per core
PIXCH = 1024
NPIX = W // PIXCH              # 2

X_PAD = np.float32(0.0)
Y_PAD = np.float32(1.5e38)
T_PAD = np.float32(-1.5e38)
IDX_PAD = np.float32(2.0e9)

def _chop(v, keep=10):
    b = v.view(np.uint32) & np.uint32(0xFFFFFFFF ^ ((1 << (23 - keep)) - 1))
    return b.view(np.float32)


def _split3(v):
    v = np.ascontiguousarray(v, np.float32)
    v0 = _chop(v)
    v1 = _chop((v - v0).astype(np.float32))
    v2 = _chop((v - v0 - v1).astype(np.float32))
    return v0, v1, v2


LAST_PROFILE = {}
LAST_DEBUG = {}
_PROGRAM_CACHE = {}


def _import_bass():
    import sys
    try:
        import concourse.bass  # noqa: F401
    except ImportError:
        for p in ("/opt/trn_rl_repo", "/root/.axon_site/_ro/trn_rl_repo"):
            if os.path.isdir(p) and p not in sys.path:
                sys.path.insert(0, p)
    import concourse.bass as bass
    import concourse.mybir as mybir
    import concourse.tile as tile
    import concourse.bacc as bacc
    return bass, mybir, tile, bacc


DVE_CHUNKS = frozenset(list(range(1, NCH, 2)) + [0])  # 7 of 13 on DVE

def _build_program(S, dyn, mm_dtype="f32r12", single_core=False):
    """Build the SPMD Bass program. S = padded survivor count (mult of 128)."""
    bass, mybir, tile, bacc = _import_bass()
    F32 = mybir.dt.float32
    BF16 = mybir.dt.bfloat16
    Alu = mybir.AluOpType
    Act = mybir.ActivationFunctionType
    AX = mybir.AxisListType

    NT = S // 128
    nc = bacc.Bacc("TRN2", target_bir_lowering=False, debug=False,
                   num_devices=1 if single_core else NCORES)

    # ---- I/O ----
    if mm_dtype == "f32r12":
        MMDT, KK = mybir.dt.float32r, 12
    else:
        MMDT, KK = F32, 3
    i_lhsT = nc.dram_tensor("lhsT", [KK, S], MMDT, kind="ExternalInput").ap()
    i_rhs = nc.dram_tensor("rhs3", [KK, PPC_PAD], MMDT, kind="ExternalInput").ap()
    i_idx = nc.dram_tensor("idxrow", [1, S], F32, kind="ExternalInput").ap()
    i_s = nc.dram_tensor("srow", [1, S], F32, kind="ExternalInput").ap()
    i_t = nc.dram_tensor("trow", [1, S], F32, kind="ExternalInput").ap()
    i_dren = nc.dram_tensor("dren", [ROWS, W], F32, kind="ExternalInput").ap()
    i_dpri = nc.dram_tensor("dpri", [ROWS, W], F32, kind="ExternalInput").ap()
    o_tinv = nc.dram_tensor("tinv", [ROWS, W], F32, kind="ExternalOutput").ap()
    o_met = nc.dram_tensor("metric", [ROWS, W], F32, kind="ExternalOutput").ap()
    o_l1p = nc.dram_tensor("l1p", [ROWS, 1], F32, kind="ExternalOutput").ap()
    o_cnt = nc.dram_tensor("cnts", [1, S], F32, kind="ExternalOutput").ap()

    cc_in = nc.dram_tensor("cc_in", [S], F32)
    cc_out = nc.dram_tensor("cc_out", [S], F32, addr_space="Shared")

    with tile.TileContext(nc) as tc:
        with (
            tc.tile_pool(name="per", bufs=1) as per,      # persistent
            tc.tile_pool(name="scr", bufs=2) as scr,      # rotating scratch
            tc.tile_pool(name="ps", bufs=3, space="PSUM") as ps,
        ):
            # ---- persistent tiles / constants ----
            lhsT = per.tile([KK, S], MMDT)
            rhs = per.tile([KK, PPC_PAD], MMDT)
            nc.sync.dma_start(lhsT[:], i_lhsT[:])
            nc.sync.dma_start(rhs[:], i_rhs[:])

            dyn_tile = per.tile([128, CH], F32)
            nc.vector.memset(dyn_tile[:], dyn)
            dyn_col = per.tile([128, 1], F32)
            nc.vector.memset(dyn_col[:], dyn)
            eps_col = per.tile([128, 1], F32)
            zero_col = per.tile([128, 1], F32)

            dren = per.tile([ROWS, W], F32)
            dpri = per.tile([ROWS, W], F32)
            tinv = per.tile([ROWS, W], F32)
            met = per.tile([ROWS, W], F32)
            nc.sync.dma_start(dren[:], i_dren[:])
            nc.sync.dma_start(dpri[:], i_dpri[:])

            # ---- count phase ----
            W2 = 2 * NCH
            cnt_all = per.tile([128, NT * W2], F32)
            sg_all = per.tile([128, NT * W2], F32)
            nc.vector.memset(cnt_all[:], 0.0)
            nc.vector.memset(sg_all[:], 0.0)
            cnt_cols = [cnt_all[:, t * W2:(t + 1) * W2] for t in range(NT)]
            sg_cols = [sg_all[:, t * W2:(t + 1) * W2] for t in range(NT)]

            # pair point-chunks: two PSUM banks per consumer op (wider ops
            # amortize the DVE/ACT per-instruction overhead)
    

            pairs = []
            ch = 0
            while ch < NCH:
                w = 2 * CH if ch + 1 < NCH else CH
                pairs.append((ch, w))
                ch += w // CH
            # DVE handles ~54% of the pairs, ACT the rest
            dve_pairs = {0, 2, 4, 6}
            n_dve_chunks = sum(pairs[p][1] // CH for p in dve_pairs)
            for pi, (ch, w) in enumerate(pairs):
                for t in range(NT):
                    r_ps = ps.tile([128, w], F32, tag="r")
                    nc.tensor.matmul(r_ps[:, 0:CH], lhsT[:, bass.ts(t, 128)],
                                     rhs[:, bass.ts(ch, CH)])
                    if w == 2 * CH:
                        nc.tensor.matmul(r_ps[:, CH:2 * CH],
                                         lhsT[:, bass.ts(t, 128)],
                                         rhs[:, bass.ts(ch + 1, CH)])
                    if pi not in dve_pairs:
                        # ACT path: net counts via the sign trick
                        sg1 = scr.tile([128, w], BF16, tag="sg")
                        nc.scalar.activation(
                            sg1[:], r_ps[:], Act.Sign, bias=dyn_col[:],
                            scale=-1.0,
                            accum_out=sg_cols[t][:, 2 * pi:2 * pi + 1])
                        sg2 = scr.tile([128, w], BF16, tag="sg")
                        nc.scalar.activation(
                            sg2[:], r_ps[:], Act.Sign, bias=dyn_col[:],
                            scale=1.0,
                            accum_out=sg_cols[t][:, 2 * pi + 1:2 * pi + 2])
                    else:
                        # DVE path: two one-sided counts (A=#(r<dyn), B=#(r>-dyn))
                        cb1 = scr.tile([128, w], BF16, tag="cb1")
                        nc.vector.tensor_scalar(
                            out=cb1[:], in0=r_ps[:], scalar1=dyn, scalar2=None,
                            op0=Alu.is_lt, op1=Alu.add,
                            accum_out=cnt_cols[t][:, 2 * pi:2 * pi + 1])
                        cb2 = scr.tile([128, w], BF16, tag="cb2")
                        nc.vector.tensor_scalar(
                            out=cb2[:], in0=r_ps[:], scalar1=-dyn, scalar2=None,
                            op0=Alu.is_gt, op1=Alu.add,
                            accum_out=cnt_cols[t][:, 2 * pi + 1:2 * pi + 2])

            # Serialize ACT table sets: pixel Ln/Exp wait on the sign phase
            # via a real data dep (eps/zero cols derive from sg_cols).
            nc.vector.tensor_scalar(zero_col[:], sg_all[:, 0:1], 0.0,
                                    None, op0=Alu.mult)
            nc.vector.tensor_scalar(eps_col[:], zero_col[:], 1e-6,
                                    None, op0=Alu.add)

            # counts = sum(cnt cols) + floor(0.5*sum(sg cols)), all NT tiles
            # vectorized as [128, NT] ops
            c1 = scr.tile([128, NT], F32, tag="c1")
            nc.vector.reduce_sum(
                c1[:], cnt_all[:].rearrange("p (t c) -> p t c", c=W2),
                axis=AX.X)
            c2 = scr.tile([128, NT], F32, tag="c2")
            nc.vector.reduce_sum(
                c2[:], sg_all[:].rearrange("p (t c) -> p t c", c=W2),
                axis=AX.X)
            hf = scr.tile([128, NT], F32, tag="hf")
            nc.vector.tensor_scalar(hf[:], c2[:], 0.5, None, op0=Alu.mult)
            # floor(hf) for hf in {k, k+0.5}: RNE((hf-0.25)+2^23)-2^23
            nc.vector.tensor_scalar(hf[:], hf[:], 8388607.75, None, op0=Alu.add)
            nc.vector.tensor_scalar(hf[:], hf[:], -8388608.0, None, op0=Alu.add)
            nc.vector.tensor_scalar(c1[:], c1[:], float(-n_dve_chunks * CH),
                                    None, op0=Alu.add)
            ct = scr.tile([128, NT], F32, tag="ct")
            nc.vector.tensor_add(ct[:], c1[:], hf[:])
            for t in range(NT):
                nc.sync.dma_start(
                    bass.AP(cc_in, t * 128, [[1, 128]]), ct[:, t:t + 1])

            # ---- all-reduce counts across the 8 cores ----
            if single_core:
                nc.gpsimd.dma_start(cc_out[:], cc_in[:])
            else:
                nc.gpsimd.collective_compute(
                    "AllReduce", Alu.add,
                    replica_groups=[list(range(NCORES))],
                    ins=[cc_in[:]],
                    outs=[cc_out[:]],
                )

            # ---- argmax (first max, original iteration order) ----
            crow = per.tile([1, S], F32)
            nc.sync.dma_start(crow[:], bass.AP(cc_out, 0, [[0, 1], [1, S]]))
            nc.sync.dma_start(o_cnt[:], crow[:])
            irow = per.tile([1, S], F32)
            srow = per.tile([1, S], F32)
            trow = per.tile([1, S], F32)
            nc.sync.dma_start(irow[:], i_idx[:])
            nc.sync.dma_start(srow[:], i_s[:])
            nc.sync.dma_start(trow[:], i_t[:])

            mx = per.tile([1, 1], F32)
            nc.vector.reduce_max(mx[:], crow[:], axis=AX.X)
            nm = scr.tile([1, S], F32, tag="nm")
            nc.vector.tensor_scalar(nm[:], crow[:], mx[:, 0:1], 4.0e9,
                                    op0=Alu.not_equal, op1=Alu.mult)
            cand = scr.tile([1, S], F32, tag="cand")
            nc.vector.tensor_add(cand[:], nm[:], irow[:])
            best = per.tile([1, 1], F32)
            nc.vector.tensor_reduce(best[:], cand[:], axis=AX.X, op=Alu.min)

            oh1 = scr.tile([1, S], F32, tag="oh1")
            s_best = per.tile([1, 1], F32)
            nc.vector.scalar_tensor_tensor(
                out=oh1[:], in0=irow[:], scalar=best[:, 0:1], in1=srow[:],
                op0=Alu.is_equal, op1=Alu.mult, accum_out=s_best[:])
            oh2 = scr.tile([1, S], F32, tag="oh2")
            t_best = per.tile([1, 1], F32)
            nc.vector.scalar_tensor_tensor(
                out=oh2[:], in0=irow[:], scalar=best[:, 0:1], in1=trow[:],
                op0=Alu.is_equal, op1=Alu.mult, accum_out=t_best[:])

            # broadcast (s,t) to all 128 partitions via a K=1 PE ones-matmul
            # (exact: single 1.0*v products, no accumulation)
            ones_f = per.tile([1, 128], F32)
            nc.vector.memset(ones_f[:], 1.0)
            ps_s = ps.tile([128, 1], F32, tag="bcs", bufs=1)
            nc.tensor.matmul(ps_s[:], ones_f[:], s_best[:])
            ps_t = ps.tile([128, 1], F32, tag="bct", bufs=1)
            nc.tensor.matmul(ps_t[:], ones_f[:], t_best[:])
            s_bc = per.tile([128, 1], F32)
            t_bc = per.tile([128, 1], F32)
            nc.vector.tensor_copy(s_bc[:], ps_s[:])
            nc.vector.tensor_copy(t_bc[:], ps_t[:])

            # ---- pixel phase ----
            # t_inv = exp(-ln(d_ren + 1e-6))   (independent of s,t)
            # phase 1: all Ln ops batched (one table set), aligned prep
    

            lndT = []
            lnmM = []
            alT = []
            for pc in range(NPIX):
                cs = bass.ts(pc, PIXCH)
                lnd = scr.tile([128, PIXCH], F32, tag=f"lndT{pc}", bufs=1,
                               name=f"lndT{pc}")
                nc.scalar.activation(lnd[:], dren[:, cs], Act.Ln,
                                     bias=eps_col[:], scale=1.0)
                lndT.append(lnd)
                v = scr.tile([128, PIXCH], F32, tag=f"alv{pc}", bufs=1,
                             name=f"alv{pc}")
                nc.vector.tensor_scalar(v[:], dpri[:, cs], s_bc[:, 0:1], None,
                                        op0=Alu.mult)
                nc.vector.tensor_scalar(v[:], v[:], t_bc[:, 0:1], None,
                                        op0=Alu.add)
                alT.append(v)
                mx2 = scr.tile([128, PIXCH], F32, tag="mx2")
                nc.vector.tensor_scalar(mx2[:], v[:], 1e-4, None, op0=Alu.max)
                lnm = scr.tile([128, PIXCH], F32, tag=f"lnmM{pc}", bufs=1,
                               name=f"lnmM{pc}")
                nc.scalar.activation(lnm[:], mx2[:], Act.Ln,
                                     bias=zero_col[:], scale=1.0)
                lnmM.append(lnm)
            # Exp ops wait for every Ln via a rebuilt bias column
            zeroE = per.tile([128, 1], F32)
            nc.vector.tensor_scalar(zeroE[:], lnmM[-1][:, 0:1], 0.0, None,
                                    op0=Alu.mult)

            l1cols = per.tile([128, NPIX], F32)
            for pc in range(NPIX):
                cs = bass.ts(pc, PIXCH)
                al = alT[pc]
                nc.scalar.activation(tinv[:, cs], lndT[pc][:], Act.Exp,
                                     bias=zeroE[:], scale=-1.0)
                nc.scalar.activation(met[:, cs], lnmM[pc][:], Act.Exp,
                                     bias=zeroE[:], scale=-1.0)
                # masked L1 partial: |al - tinv| * (dren>0.1) * (dren<100)
                dm = scr.tile([128, PIXCH], F32, tag="dm")
                nc.gpsimd.tensor_tensor(dm[:], al[:], tinv[:, cs],
                                        op=Alu.subtract)
                ab = dm
                I32 = mybir.dt.int32
                nc.vector.tensor_scalar(ab[:].bitcast(I32), dm[:].bitcast(I32),
                                        0x7FFFFFFF, None, op0=Alu.bitwise_and)
                m1 = scr.tile([128, PIXCH], F32, tag="m1")
                nc.vector.scalar_tensor_tensor(
                    out=m1[:], in0=dren[:, cs], scalar=0.1, in1=ab[:],
                    op0=Alu.is_gt, op1=Alu.mult)
                m2 = scr.tile([128, PIXCH], F32, tag="m2")
                nc.vector.scalar_tensor_tensor(
                    out=m2[:], in0=dren[:, cs], scalar=100.0, in1=m1[:],
                    op0=Alu.is_lt, op1=Alu.mult,
                    accum_out=l1cols[:, pc:pc + 1])
            nc.sync.dma_start(o_tinv[:], tinv[:])

            nc.sync.dma_start(o_met[:], met[:])
            l1p = per.tile([128, 1], F32)
            nc.vector.reduce_sum(l1p[:], l1cols[:], axis=AX.X)
            nc.sync.dma_start(o_l1p[:], l1p[:])

    nc.compile()
    return nc


def _host_control(rendered_depth, prior_disparity):
    """Exact host-side replication of the data-dependent control scalars."""
    import jax
    import jax.numpy as jnp
    cpu = jax.devices("cpu")[0]

    d_ren = np.asarray(rendered_depth, dtype=np.float32)
    d_pri = np.asarray(prior_disparity, dtype=np.float32)
    mask = (d_ren > 0.1) & (d_ren < 100.0) & np.isfinite(d_ren)
    mask_flat = mask.reshape(-1)
    num_pts = int(mask_flat.sum())
    n_valid = num_pts  # same expression in the reference
    P = d_ren.size

    x = d_pri.reshape(-1).astype(np.float32)
    y = (np.float32(1.0) / (d_ren.reshape(-1).astype(np.float32) + np.float32(1e-6)))

    idx_m = np.flatnonzero(mask_flat)
    if idx_m.size < P:
        idx_m = np.concatenate([idx_m, np.zeros(P - idx_m.size, idx_m.dtype)])

    with jax.default_device(cpu):
        k_pair, k_sub = jax.random.split(jax.random.key(42))
        pos = np.asarray(jax.random.randint(k_pair, (ITERATIONS, 2), 0, num_pts))
        spos = np.asarray(jax.random.randint(k_sub, (SUB_N,), 0, num_pts))
        y_nan = jnp.where(jnp.asarray(mask_flat), jnp.asarray(y), jnp.nan)
        med = jnp.nanmedian(y_nan)
        dynj = jnp.nanmedian(jnp.abs(y_nan - med)) * 0.5
        dynj = jnp.where(dynj < 1e-5, THRESH, dynj)
        dyn = np.float32(np.asarray(dynj))

    pi = idx_m[pos]
    x1, x2 = x[pi[:, 0]], x[pi[:, 1]]
    y1, y2 = y[pi[:, 0]], y[pi[:, 1]]
    scales = ((y2 - y1) / ((x2 - x1) + np.float32(1e-8))).astype(np.float32)
    shifts = (y1 - scales * x1).astype(np.float32)

    si = idx_m[spos]
    x_sub = x[si].astype(np.float32)
    y_sub = y[si].astype(np.float32)

    surv = np.flatnonzero(scales > 0)
    fallback = (num_pts < 10) or (surv.size == 0)
    if fallback:
        surv_s = np.ones(1, np.float32)
        surv_t = np.zeros(1, np.float32)
        surv_i = np.zeros(1, np.float32)
    else:
        surv_s = scales[surv].astype(np.float32)
        surv_t = shifts[surv].astype(np.float32)
        surv_i = surv.astype(np.float32)

    return dict(d_ren=d_ren, d_pri=d_pri, num_pts=num_pts, n_valid=n_valid,
                dyn=dyn, scales=scales, shifts=shifts, x_sub=x_sub,
                y_sub=y_sub, surv_s=surv_s, surv_t=surv_t, surv_i=surv_i,
                fallback=fallback)


def _prepare(rendered_depth, prior_disparity, mm_dtype="f32r"):
    """Host control + program build + per-core input maps."""
    hc = _host_control(rendered_depth, prior_disparity)
    return _make_inputs(hc, mm_dtype) + (hc,)


def _make_inputs(hc, mm_dtype="f32r12"):
    dyn = float(hc["dyn"])

    # pad survivors to a multiple of 128
    S_real = hc["surv_s"].size
    NT = max(1, (S_real + 127) // 128)
    S = NT * 128
    s_full = np.ones(S, np.float32)
    t_full = np.full(S, T_PAD, np.float32)
    s_full[:S_real] = hc["surv_s"]
    t_full[:S_real] = hc["surv_t"]
    if mm_dtype == "f32r12":
        s0, s1, s2 = _split3(s_full)
        t0, t1, t2 = _split3(t_full)
        mone = -np.ones(S, np.float32)
        lhsT = np.stack([s0, s0, s1, s1, s0, s2, t0, t1, t2, mone, mone, mone])
    else:
        lhsT = np.stack([s_full, t_full, -np.ones(S, np.float32)])
    idxrow = np.full((1, S), IDX_PAD, np.float32)
    idxrow[0, :S_real] = hc["surv_i"]
    srow = np.ones((1, S), np.float32)
    srow[0, :S_real] = hc["surv_s"]
    trow = np.zeros((1, S), np.float32)
    trow[0, :S_real] = hc["surv_t"]

    key = (S, float(dyn), mm_dtype)
    if key not in _PROGRAM_CACHE:
        _PROGRAM_CACHE[key] = _build_program(S, dyn, mm_dtype)
    nc = _PROGRAM_CACHE[key]

    # per-core inputs
    in_maps = []
    for c in range(NCORES):
        xs = np.full(PPC_PAD, X_PAD, np.float32)
        ys = np.full(PPC_PAD, Y_PAD, np.float32)
        xs[:PPC] = hc["x_sub"][c * PPC:(c + 1) * PPC]
        ys[:PPC] = hc["y_sub"][c * PPC:(c + 1) * PPC]
        one = np.ones(PPC_PAD, np.float32)
        if mm_dtype == "f32r12":
            x0, x1, x2 = _split3(xs)
            y0, y1, y2 = _split3(ys)
            rhs3 = np.stack([x0, x1, x0, x1, x2, x0, one, one, one, y0, y1, y2])
        else:
            rhs3 = np.stack([xs, one, ys])
        rs = slice(c * ROWS, (c + 1) * ROWS)
        in_maps.append({
            "lhsT": lhsT, "rhs3": rhs3, "idxrow": idxrow,
            "srow": srow, "trow": trow,
            "dren": np.ascontiguousarray(hc["d_ren"][rs]),
            "dpri": np.ascontiguousarray(hc["d_pri"][rs]),
        })
    return nc, in_maps


def _finalize(results, hc):
    t_inv = np.concatenate([results[c]["tinv"] for c in range(NCORES)], axis=0)
    metric = np.concatenate([results[c]["metric"] for c in range(NCORES)], axis=0)
    l1_sum = float(sum(results[c]["l1p"].astype(np.float64).sum()
                       for c in range(NCORES)))
    n_valid = hc["n_valid"]
    l1 = l1_sum / max(n_valid, 1)
    total = np.float32(LAMBDA_L1 * l1)
    if n_valid < 100:
        total = np.float32(0.0)
    LAST_DEBUG.clear()
    LAST_DEBUG["cnts"] = results[0]["cnts"][0]
    LAST_DEBUG["hc"] = hc
    return total, t_inv, metric


def _exact_argmax_host(hc):
    """Reference-exact RANSAC winner, computed on host (fallback path)."""
    scales, shifts = hc["scales"], hc["shifts"]
    x_sub, y_sub, dyn = hc["x_sub"], hc["y_sub"], np.float32(hc["dyn"])
    counts = np.zeros(ITERATIONS, np.int64)
    CHh = 64
    for i0 in range(0, ITERATIONS, CHh):
        ss = scales[i0:i0 + CHh, None]
        tt = shifts[i0:i0 + CHh, None]
        w = ((ss * x_sub[None, :]).astype(np.float32) + tt).astype(np.float32)
        res = np.abs((w - y_sub[None, :]).astype(np.float32))
        counts[i0:i0 + CHh] = (res < dyn).sum(axis=1)
    cm = np.where(scales > 0, counts, -1)
    best = int(np.argmax(cm))
    if cm[best] >= 0 and hc["num_pts"] >= 10:
        return np.float32(scales[best]), np.float32(shifts[best])
    return np.float32(1.0), np.float32(0.0)


MARGIN_MIN = 64.0


def kernel(rendered_depth, prior_disparity):
    _import_bass()
    from concourse.bass_utils import run_bass_kernel_spmd

    hc = _host_control(rendered_depth, prior_disparity)
    nc, in_maps = _make_inputs(hc, "f32r12")
    trace = bool(os.environ.get("DEPTH_KERNEL_TRACE"))
    if trace:
        try:
            from antenv.axon_hooks import get_axon_ntff_profile_hook  # noqa: F401
        except ImportError:
            trace = False
    res = run_bass_kernel_spmd(nc, in_maps, list(range(NCORES)), trace=trace)
    LAST_PROFILE.clear()
    LAST_PROFILE["exec_time_ns"] = res.exec_time_ns
    LAST_PROFILE["res"] = res

    # Robustness certificate: the fp32r count matrix can deviate from the
    # reference-exact counts by a few units per iteration near the threshold.
    # If the observed top-2 margin is not comfortably larger than that, fall
    # back to the host-exact winner and rerun the (cheap) pixel phase with a
    # single forced candidate.
    S_real = int(hc["surv_s"].size)
    if not hc["fallback"] and S_real > 1:
        cnts = np.sort(res.results[0]["cnts"][0, :S_real])[::-1]
        margin_ok = bool(np.isfinite(cnts).all()) and \
            float(cnts[0] - cnts[1]) >= MARGIN_MIN
        if not margin_ok:
            s_b, t_b = _exact_argmax_host(hc)
            hc = dict(hc)
            hc["surv_s"] = np.array([s_b], np.float32)
            hc["surv_t"] = np.array([t_b], np.float32)
            hc["surv_i"] = np.zeros(1, np.float32)
            nc, in_maps = _make_inputs(hc, "f32r12")
            res = run_bass_kernel_spmd(nc, in_maps, list(range(NCORES)),
                                       trace=trace)
    return _finalize(res.results, hc)
